# revision 10
# baseline (speedup 1.0000x reference)
"""Trainium2 Bass kernel for nn_LocalGlobalVideoTextInteractions.

Data-parallel over batch: B=16 across 8 NeuronCores (2 batches/core).
All activations are kept in transposed layout fT[d, l] (feature on the
partition dim) so every projection is a natural PE matmul; attention scores
are computed directly in transposed form sT[m, l], softmax denominators come
from a ones-column appended to the value matrix, and normalization is applied
via a K=1 broadcast matmul + vector ops.

Matmul operand dtype is switchable: float32 (exact, 4 cyc/row) or float32r
(tf32-like, 1 cyc/row at N>=256).
"""
import os
import sys

import numpy as np

for _p in ("/opt/trn_rl_repo", "/root/.axon_site/_ro/trn_rl_repo"):
    if os.path.isdir(_p) and _p not in sys.path:
        sys.path.append(_p)

import concourse.bass as bass
import concourse.tile as tile
from concourse import bacc, mybir
from concourse import bass_utils

F32 = mybir.dt.float32
AF = mybir.ActivationFunctionType

B, L, D = 16, 512, 512
NENT, NH, DH = 3, 8, 64
AH = 256
CORES = 8
BPC = B // CORES  # batches per core
P = 128
NC = D // P  # 4 chunks
KW = 7  # band half-width (ksize 15, dilation 1)
NEG = -1e9

# MM_DT: dtype of every SBUF tensor that feeds the PE array.
MM_DT = mybir.dt.float32r if os.environ.get("KERNEL_F32R", "1") == "1" else F32
BANDED = os.environ.get("KERNEL_BANDED", "0") == "1"  # (banded local path, v2)


def _band_mask8() -> np.ndarray:
    """Additive pre-scale band mask, transposed orientation: tile c covers
    m in [128c, 128c+128) on partitions, l in [0, 512) on free dim.
    0 in band, -8e9 (= 8 * -1e9, pre-softmax-scale) outside."""
    m = (np.arange(NC)[:, None, None] * P + np.arange(P)[None, :, None])
    l = np.arange(L)[None, None, :]
    return np.where(np.abs(l - m) <= KW, 0.0, 8.0 * NEG).astype(np.float32)


def _build():
    nc = bacc.Bacc("TRN2", target_bir_lowering=False, debug=False)

    def din(name, shape, dt=F32):
        return nc.dram_tensor(name, shape, dt, kind="ExternalInput").ap()

    seg_feats = din("seg_feats", [BPC, L, D])
    seg_masks = din("seg_masks", [BPC, L])
    se_feats = din("se_feats", [BPC, NENT, D], MM_DT)
    hp_w1 = din("hp_w1", [NENT, D, D], MM_DT)
    hp_b1 = din("hp_b1", [NENT, D])
    hp_w2 = din("hp_w2", [NENT, D, D], MM_DT)
    hp_b2 = din("hp_b2", [NENT, D])
    hp_w3 = din("hp_w3", [NENT, D, D], MM_DT)
    hp_b3 = din("hp_b3", [NENT, D])
    local_cW = din("local_cW", [NENT, 2, D, 2 * D], MM_DT)
    local_cb = din("local_cb", [NENT, 2, 2 * D])
    local_vW = din("local_vW", [NENT, 2, D, D], MM_DT)
    local_vb = din("local_vb", [NENT, 2, D])
    global_cW = din("global_cW", [2, D, 2 * D], MM_DT)
    global_cb = din("global_cb", [2, 2 * D])
    global_vW = din("global_vW", [2, D, D], MM_DT)
    global_vb = din("global_vb", [2, D])
    satt_w1 = din("satt_w1", [D, AH], MM_DT)
    satt_w2 = din("satt_w2", [AH, 1], MM_DT)
    band_mask8 = din("band_mask8", [NC, P, L])

    out_a = nc.dram_tensor("out_a", [BPC, L, D], F32, kind="ExternalOutput").ap()
    out_sattw = nc.dram_tensor("out_sattw", [BPC, NENT], F32, kind="ExternalOutput").ap()

    with tile.TileContext(nc) as tc:
        _emit(nc, tc, locals())
    nc.compile()
    return nc


def _autoname(pool):
    orig = pool.tile

    def tile(shape, dtype, **kw):
        if "name" not in kw:
            kw["name"] = kw.get("tag") or "tmp"
        return orig(shape, dtype, **kw)

    pool.tile = tile
    return pool


def _emit(nc, tc, t):
    from contextlib import ExitStack

    ctx = ExitStack()
    const = ctx.enter_context(tc.tile_pool(name="const", bufs=1))
    wpool = ctx.enter_context(tc.tile_pool(name="wpool", bufs=2))
    bpool = ctx.enter_context(tc.tile_pool(name="bpool", bufs=2))
    fpool = ctx.enter_context(tc.tile_pool(name="fpool", bufs=1))
    apool = ctx.enter_context(tc.tile_pool(name="apool", bufs=1))
    spool = ctx.enter_context(tc.tile_pool(name="spool", bufs=1))  # segT
    hpool = ctx.enter_context(tc.tile_pool(name="hpool", bufs=1))  # h1
    kpool = ctx.enter_context(tc.tile_pool(name="kpool", bufs=1))  # mk/mq
    vpool = ctx.enter_context(tc.tile_pool(name="vpool", bufs=1))  # mv_aug
    epool = ctx.enter_context(tc.tile_pool(name="epool", bufs=2))  # exp(scores)
    rpool = ctx.enter_context(tc.tile_pool(name="rpool", bufs=2))  # resid temps
    npool = ctx.enter_context(tc.tile_pool(name="npool", bufs=2))  # nat in/out

    ps_proj = ctx.enter_context(tc.tile_pool(name="ps_proj", bufs=2, space="PSUM"))
    ps_score = ctx.enter_context(tc.tile_pool(name="ps_score", bufs=2, space="PSUM"))
    ps_av = ctx.enter_context(tc.tile_pool(name="ps_av", bufs=2, space="PSUM"))
    ps_bc = ctx.enter_context(tc.tile_pool(name="ps_bc", bufs=2, space="PSUM"))

    for _pl in (const, wpool, bpool, fpool, apool, spool, hpool, kpool, vpool,
                epool, rpool, npool, ps_proj, ps_score, ps_av, ps_bc):
        _autoname(_pl)

    mm = nc.tensor.matmul

    # ---- constants ----
    ones_mat = const.tile([P, P], MM_DT, tag="ones")
    nc.vector.memset(ones_mat.bitcast(F32), 1.0)
    ident = const.tile([P, P], F32, tag="ident")
    from concourse.masks import make_identity
    make_identity(nc, ident)
    if MM_DT != F32:
        ident_mm = const.tile([P, P], MM_DT, tag="identmm")
        nc.vector.tensor_copy(ident_mm, ident)
    else:
        ident_mm = ident

    band_sb = [const.tile([P, L], F32, tag=f"band{c}") for c in range(NC)]
    for c in range(NC):
        nc.sync.dma_start(out=band_sb[c], in_=t["band_mask8"][c])

    # seg mask -> additive bias per m-partition: (mask-1)*1e9
    negb = const.tile([P, 1], F32, tag="negb")
    nc.vector.memset(negb, NEG)
    mbias = [[const.tile([P, 1], F32, tag=f"mb{b}{c}") for c in range(NC)]
             for b in range(BPC)]
    for b in range(BPC):
        for c in range(NC):
            nc.sync.dma_start(out=mbias[b][c], in_=t["seg_masks"][b, c * P:(c + 1) * P])
            nc.scalar.activation(out=mbias[b][c], in_=mbias[b][c], func=AF.Identity,
                                 scale=-NEG, bias=negb)

    # seT[ki][p, b, n] = se_feats[b, n, 128ki + p]
    seT = [const.tile([P, BPC, NENT], MM_DT, tag=f"seT{ki}") for ki in range(NC)]
    for ki in range(NC):
        ap = bass.AP(tensor=t["se_feats"].tensor, offset=ki * P,
                     ap=[[1, P], [NENT * D, BPC], [D, NENT]])
        nc.sync.dma_start(out=seT[ki], in_=ap)

    # ---- attentive pooling (se_feats only) -> sattwB [128, (b,n)] ----
    sw1 = [const.tile([P, AH], MM_DT, tag=f"sw1{ki}") for ki in range(NC)]
    for ki in range(NC):
        nc.sync.dma_start(out=sw1[ki], in_=t["satt_w1"][ki * P:(ki + 1) * P, :])
    sw2 = [const.tile([P, 1], MM_DT, tag=f"sw2{a}") for a in range(AH // P)]
    for a in range(AH // P):
        nc.sync.dma_start(out=sw2[a], in_=t["satt_w2"][a * P:(a + 1) * P, :])

    th = [const.tile([P, BPC * NENT], MM_DT, tag=f"th{a}") for a in range(AH // P)]
    for a in range(AH // P):
        ps = ps_proj.tile([P, BPC * NENT], F32, tag="proj")
        for ki in range(NC):
            mm(ps, sw1[ki][:, a * P:(a + 1) * P],
               seT[ki].rearrange("p b n -> p (b n)"),
               start=(ki == 0), stop=(ki == NC - 1))
        nc.scalar.activation(out=th[a], in_=ps, func=AF.Tanh)
    ps_alpha = ps_av.tile([1, BPC * NENT], F32, tag="av")
    for a in range(AH // P):
        mm(ps_alpha, sw2[a], th[a], start=(a == 0), stop=(a == AH // P - 1))
    ealpha = const.tile([1, BPC * NENT], F32, tag="ealpha")
    # magnitudes are tiny: skip max-subtraction in these softmaxes
    nc.scalar.activation(out=ealpha, in_=ps_alpha, func=AF.Exp)
    asum = const.tile([1, BPC], F32, tag="asum")
    nc.vector.reduce_sum(out=asum, in_=ealpha.rearrange("o (b n) -> o b n", b=BPC),
                         axis=mybir.AxisListType.X)
    arecip = const.tile([1, BPC], F32, tag="arecip")
    nc.vector.reciprocal(arecip, asum)
    sattw = const.tile([1, BPC * NENT], MM_DT, tag="sattw")
    for b in range(BPC):
        nc.vector.tensor_scalar_mul(sattw[:, b * NENT:(b + 1) * NENT],
                                    ealpha[:, b * NENT:(b + 1) * NENT],
                                    arecip[:, b:b + 1])
    nc.gpsimd.dma_start(out=t["out_sattw"].rearrange("b n -> (b n)")[None, :], in_=sattw)
    ps_sw = ps_bc.tile([P, BPC * NENT], F32, tag="bc")
    mm(ps_sw, ones_mat[0:1, :], sattw, start=True, stop=True)
    sattwB = const.tile([P, BPC * NENT], F32, tag="sattwB")
    nc.scalar.copy(sattwB, ps_sw)

    # ---- h2[n] = relu(se_feats[:, n] @ hp_w2[n] + hp_b2[n]), all (b, n) ----
    h2T = [[const.tile([P, BPC], F32, tag=f"h2{n}{c}") for c in range(NC)]
           for n in range(NENT)]
    for n in range(NENT):
        w2 = [wpool.tile([P, D], MM_DT, tag=f"w1_{ki}", bufs=1) for ki in range(NC)]
        for ki in range(NC):
            nc.sync.dma_start(out=w2[ki], in_=t["hp_w2"][n, ki * P:(ki + 1) * P, :])
        for c in range(NC):
            b2 = bpool.tile([P, 1], F32, tag="b2")
            nc.sync.dma_start(out=b2, in_=t["hp_b2"][n, c * P:(c + 1) * P])
            ps = ps_proj.tile([P, BPC], F32, tag="proj")
            for ki in range(NC):
                mm(ps, w2[ki][:, c * P:(c + 1) * P], seT[ki][:, :, n],
                   start=(ki == 0), stop=(ki == NC - 1))
            nc.scalar.activation(out=h2T[n][c], in_=ps, func=AF.Relu, bias=b2)

    # ---- transpose seg_feats -> segT[b][dc] [128(d), 512(l)] ----
    segT = [[spool.tile([P, L], MM_DT, tag=f"seg{b}{dc}") for dc in range(NC)]
            for b in range(BPC)]
    for b in range(BPC):
        for lc in range(NC):
            nat = npool.tile([P, D], F32, tag="nat", bufs=1)
            nc.sync.dma_start(out=nat, in_=t["seg_feats"][b, lc * P:(lc + 1) * P, :])
            for dc in range(NC):
                pst = ps_score.tile([P, P], F32, tag="score")
                nc.tensor.transpose(pst, nat[:, dc * P:(dc + 1) * P], ident)
                nc.scalar.copy(segT[b][dc][:, lc * P:(lc + 1) * P], pst)

    fT = [[fpool.tile([P, L], MM_DT, tag=f"f{b}{c}") for c in range(NC)]
          for b in range(BPC)]
    aT = [[apool.tile([P, L], MM_DT, tag=f"a{b}{c}") for c in range(NC)]
          for b in range(BPC)]

    def load_biases(src_row, tag):
        """4x [128,1] f32 bias tiles from a length-512 DRAM row."""
        tiles = []
        for c in range(NC):
            bt = bpool.tile([P, 1], F32, tag=f"{tag}{c}")
            nc.sync.dma_start(out=bt, in_=src_row[c * P:(c + 1) * P])
            tiles.append(bt)
        return tiles

    def nl_block(b, xT, cw, vw, cbq, cbv_row, vb, banded):
        """One non-local block, in-place residual update of xT (4 tiles)."""
        mk = [kpool.tile([P, L], MM_DT, tag=f"mk{c}") for c in range(NC)]
        mq = [kpool.tile([P, L], MM_DT, tag=f"mq{c}") for c in range(NC)]
        for c in range(NC):
            ps = ps_proj.tile([P, L], F32, tag="proj")
            for ki in range(NC):
                mm(ps, vw[ki][:, c * P:(c + 1) * P], xT[ki],
                   start=(ki == 0), stop=(ki == NC - 1))
            nc.scalar.activation(out=mk[c], in_=ps, func=AF.Identity, bias=vb[c])
            ps = ps_proj.tile([P, L], F32, tag="proj")
            for ki in range(NC):
                mm(ps, cw[ki][:, c * P:(c + 1) * P], xT[ki],
                   start=(ki == 0), stop=(ki == NC - 1))
            nc.scalar.activation(out=mq[c], in_=ps, func=AF.Identity, bias=cbq[c])
        # mv in natural layout [m, dv], embedded as the stationary operand of
        # the AV matmul so each head's result lands on its own psum partitions
        # (trn2 matmul output always starts at partition 0): even heads use
        # columns 0:64 + ones at 64; odd heads columns 64:128 + ones at 32.
        # Unused columns are zero.
        mv = [vpool.tile([P, NH, P], MM_DT, tag=f"mv{c}") for c in range(NC)]
        for c in range(NC):
            ps = ps_proj.tile([P, L], F32, tag="proj")
            for ki in range(NC):
                mm(ps, xT[ki][:, c * P:(c + 1) * P], cw[ki][:, D:2 * D],
                   start=(ki == 0), stop=False)
            mm(ps, ones_mat[0:1, :], cbv_row, start=False, stop=True)
            psv = ps[:].rearrange("p (h d) -> p h d", h=NH)
            nc.gpsimd.memset(mv[c].bitcast(F32), 0.0)
            nc.vector.tensor_copy(mv[c][:, 0::2, 0:DH], psv[:, 0::2, :])
            nc.vector.tensor_copy(mv[c][:, 1::2, DH:P], psv[:, 1::2, :])
            nc.gpsimd.memset(mv[c][:, 0::2, DH:DH + 1].bitcast(F32), 1.0)
            nc.gpsimd.memset(mv[c][:, 1::2, 32:33].bitcast(F32), 1.0)
        for h in range(NH):
            j, base = h // 2, (h % 2) * DH
            eT = []
            for c in range(NC):
                ps = ps_score.tile([P, L], F32, tag="score")
                mm(ps, mq[j][base:base + DH, c * P:(c + 1) * P],
                   mk[j][base:base + DH, :], start=True, stop=True)
                if banded:
                    nc.vector.tensor_add(out=ps, in0=ps, in1=band_sb[c])
                e = epool.tile([P, L], MM_DT, tag=f"e{c}")
                nc.scalar.activation(out=e, in_=ps, func=AF.Exp,
                                     scale=1.0 / np.sqrt(DH), bias=mbias[b][c])
                eT.append(e)
            # AV: the head-parity column placement in mv makes rT land on
            # the head's own partitions; denominator on a spare aligned row.
            psr = ps_av.tile([P, L], F32, tag="av")
            for c in range(NC):
                mm(psr, mv[c][:, h, :], eT[c], start=(c == 0), stop=(c == NC - 1))
            row = DH if h % 2 == 0 else 32
            rt = psr[base:base + DH, :]
            rc = rpool.tile([P, L], MM_DT, tag="recip")
            with nc.allow_low_precision(reason="tf32 softmax denom"):
                nc.vector.reciprocal(rc[row:row + 1, :], psr[row:row + 1, :])
            psb = ps_bc.tile([P, L], F32, tag="bc")
            mm(psb, ones_mat[row:row + 1, :], rc[row:row + 1, :],
               start=True, stop=True)
            rb = rpool.tile([P, L], MM_DT, tag="rb", bufs=1)
            nc.scalar.copy(rb[base:base + DH, :], psb[base:base + DH, :])
            tt = rpool.tile([P, L], MM_DT, tag="t", bufs=1)
            nc.vector.tensor_mul(tt[base:base + DH, :], rt, rb[base:base + DH, :])
            nc.vector.tensor_add(out=xT[j][base:base + DH, :],
                                 in0=xT[j][base:base + DH, :],
                                 in1=tt[base:base + DH, :])

    # ---- main pipeline over entities ----
    for n in range(NENT):
        w1 = [wpool.tile([P, D], MM_DT, tag=f"w1_{ki}", bufs=1) for ki in range(NC)]
        w3 = [wpool.tile([P, D], MM_DT, tag=f"w3_{ki}", bufs=1) for ki in range(NC)]
        for ki in range(NC):
            nc.sync.dma_start(out=w1[ki], in_=t["hp_w1"][n, ki * P:(ki + 1) * P, :])
            nc.sync.dma_start(out=w3[ki], in_=t["hp_w3"][n, ki * P:(ki + 1) * P, :])
        b1 = load_biases(t["hp_b1"][n], "b1")
        b3 = load_biases(t["hp_b3"][n], "b3")
        for b in range(BPC):
            h1 = [hpool.tile([P, L], MM_DT, tag=f"h1{c}") for c in range(NC)]
            for c in range(NC):
                ps = ps_proj.tile([P, L], F32, tag="proj")
                for ki in range(NC):
                    mm(ps, w1[ki][:, c * P:(c + 1) * P], segT[b][ki],
                       start=(ki == 0), stop=(ki == NC - 1))
                nc.scalar.activation(out=h1[c], in_=ps, func=AF.Relu, bias=b1[c])
                nc.vector.tensor_scalar_mul(h1[c], h1[c], h2T[n][c][:, b:b + 1])
            for c in range(NC):
                ps = ps_proj.tile([P, L], F32, tag="proj")
                for ki in range(NC):
                    mm(ps, w3[ki][:, c * P:(c + 1) * P], h1[ki],
                       start=(ki == 0), stop=(ki == NC - 1))
                nc.scalar.activation(out=fT[b][c], in_=ps, func=AF.Relu, bias=b3[c])
        for s in range(2):
            cw = [wpool.tile([P, 2 * D], MM_DT, tag=f"cw{ki}") for ki in range(NC)]
            vw = [wpool.tile([P, D], MM_DT, tag=f"vw{ki}") for ki in range(NC)]
            for ki in range(NC):
                nc.sync.dma_start(out=cw[ki], in_=t["local_cW"][n, s, ki * P:(ki + 1) * P, :])
                nc.sync.dma_start(out=vw[ki], in_=t["local_vW"][n, s, ki * P:(ki + 1) * P, :])
            cbq = load_biases(t["local_cb"][n, s, 0:D], "cbq")
            vb = load_biases(t["local_vb"][n, s], "vb")
            cbv_row = bpool.tile([1, D], MM_DT, tag="cbv")
            nc.gpsimd.dma_start(out=cbv_row, in_=t["local_cb"][n, s, D:2 * D][None, :])
            for b in range(BPC):
                nl_block(b, fT[b], cw, vw, cbq, cbv_row, vb, banded=True)
        # accumulate entity result into aT, weighted by sattw[b, n]
        for b in range(BPC):
            w_ap = sattwB[:, b * NENT + n:b * NENT + n + 1]
            for c in range(NC):
                if n == 0:
                    nc.vector.tensor_scalar_mul(aT[b][c], fT[b][c], w_ap)
                else:
                    wt = rpool.tile([P, L], MM_DT, tag="wt", bufs=1)
                    nc.scalar.activation(out=wt, in_=fT[b][c], func=AF.Copy, scale=w_ap)
                    nc.vector.tensor_add(out=aT[b][c], in0=aT[b][c], in1=wt)

    # ---- global blocks ----
    for s in range(2):
        cw = [wpool.tile([P, 2 * D], MM_DT, tag=f"cw{ki}") for ki in range(NC)]
        vw = [wpool.tile([P, D], MM_DT, tag=f"vw{ki}") for ki in range(NC)]
        for ki in range(NC):
            nc.sync.dma_start(out=cw[ki], in_=t["global_cW"][s, ki * P:(ki + 1) * P, :])
            nc.sync.dma_start(out=vw[ki], in_=t["global_vW"][s, ki * P:(ki + 1) * P, :])
        cbq = load_biases(t["global_cb"][s, 0:D], "cbq")
        vb = load_biases(t["global_vb"][s], "vb")
        cbv_row = bpool.tile([1, D], MM_DT, tag="cbv")
        nc.gpsimd.dma_start(out=cbv_row, in_=t["global_cb"][s, D:2 * D][None, :])
        for b in range(BPC):
            nl_block(b, aT[b], cw, vw, cbq, cbv_row, vb, banded=False)

    # ---- transpose back and write out ----
    for b in range(BPC):
        for lc in range(NC):
            out_nat = npool.tile([P, D], F32, tag="on", bufs=1)
            for dc in range(NC):
                pst = ps_score.tile([P, P], MM_DT, tag="score")
                nc.tensor.transpose(pst, aT[b][dc][:, lc * P:(lc + 1) * P], ident_mm)
                nc.scalar.copy(out_nat[:, dc * P:(dc + 1) * P], pst)
            nc.sync.dma_start(out=t["out_a"][b, lc * P:(lc + 1) * P, :], in_=out_nat)

    ctx.close()


_NC_CACHE = None


def kernel(**inputs):
    global _NC_CACHE
    if _NC_CACHE is None:
        _NC_CACHE = _build()
    nc = _NC_CACHE

    band = _band_mask8()
    shared = {k: np.ascontiguousarray(np.asarray(inputs[k], dtype=np.float32))
              for k in ("hp_w1", "hp_b1", "hp_w2", "hp_b2", "hp_w3", "hp_b3",
                        "local_cW", "local_cb", "local_vW", "local_vb",
                        "global_cW", "global_cb", "global_vW", "global_vb",
                        "satt_w1", "satt_w2")}
    seg_feats = np.asarray(inputs["seg_feats"], dtype=np.float32)
    seg_masks = np.asarray(inputs["seg_masks"], dtype=np.float32)
    se_feats = np.asarray(inputs["se_feats"], dtype=np.float32)

    in_maps = []
    for c in range(CORES):
        sl = slice(c * BPC, (c + 1) * BPC)
        m = dict(shared)
        m["seg_feats"] = np.ascontiguousarray(seg_feats[sl])
        m["seg_masks"] = np.ascontiguousarray(seg_masks[sl])
        m["se_feats"] = np.ascontiguousarray(se_feats[sl])
        m["band_mask8"] = band
        in_maps.append(m)

    res = bass_utils.run_bass_kernel_spmd(nc, in_maps, core_ids=list(range(CORES)))
    a = np.concatenate([res.results[c]["out_a"] for c in range(CORES)], axis=0)
    sattw = np.concatenate([res.results[c]["out_sattw"] for c in range(CORES)], axis=0)
    return a, sattw


# revision 18
# speedup vs baseline: 1.2890x; 1.2890x over previous
"""Trainium2 Bass kernel for nn_LocalGlobalVideoTextInteractions.

Data-parallel over batch: B=16 across 8 NeuronCores (2 batches/core).
All activations are kept in transposed layout fT[d, l] (feature on the
partition dim) so every projection is a natural PE matmul; attention scores
are computed directly in transposed form sT[m, l], softmax denominators come
from a ones-column appended to the value matrix, and normalization is applied
via a K=1 broadcast matmul + vector ops.

Matmul operand dtype is switchable: float32 (exact, 4 cyc/row) or float32r
(tf32-like, 1 cyc/row at N>=256).
"""
import os
import sys

import numpy as np

for _p in ("/opt/trn_rl_repo", "/root/.axon_site/_ro/trn_rl_repo"):
    if os.path.isdir(_p) and _p not in sys.path:
        sys.path.append(_p)

import concourse.bass as bass
import concourse.tile as tile
from concourse import bacc, mybir
from concourse import bass_utils

F32 = mybir.dt.float32
AF = mybir.ActivationFunctionType

B, L, D = 16, 512, 512
NENT, NH, DH = 3, 8, 64
AH = 256
CORES = 8
BPC = B // CORES  # batches per core
P = 128
NC = D // P  # 4 chunks
KW = 7  # band half-width (ksize 15, dilation 1)
NEG = -1e9

# MM_DT: dtype of every SBUF tensor that feeds the PE array.
MM_DT = mybir.dt.float32r if os.environ.get("KERNEL_F32R", "1") == "1" else F32
BANDED = os.environ.get("KERNEL_BANDED", "0") == "1"  # (banded local path, v2)


def _band_mask8() -> np.ndarray:
    """Additive pre-scale band mask, transposed orientation: tile c covers
    m in [128c, 128c+128) on partitions, l in [0, 512) on free dim.
    0 in band, -8e9 (= 8 * -1e9, pre-softmax-scale) outside."""
    m = (np.arange(NC)[:, None, None] * P + np.arange(P)[None, :, None])
    l = np.arange(L)[None, None, :]
    return np.where(np.abs(l - m) <= KW, 0.0, 8.0 * NEG).astype(np.float32)


def _sel_gmat():
    gmat = np.zeros((P, 2 * P), np.float32)
    gmat[DH, P] = 1.0
    gmat[32, P] = 1.0
    sel = np.zeros((NH // 2, NH, P), np.float32)
    for jj in range(NH // 2):
        sel[jj, 2 * jj, 0:DH] = 1.0
        sel[jj, 2 * jj + 1, DH:P] = 1.0
    return gmat, sel


def _build():
    nc = bacc.Bacc("TRN2", target_bir_lowering=False, debug=False)

    def din(name, shape, dt=F32):
        return nc.dram_tensor(name, shape, dt, kind="ExternalInput").ap()

    seg_feats = din("seg_feats", [BPC, L, D])
    seg_masks = din("seg_masks", [BPC, L])
    se_feats = din("se_feats", [BPC, NENT, D], MM_DT)
    hp_w1 = din("hp_w1", [NENT, D, D], MM_DT)
    hp_b1 = din("hp_b1", [NENT, D])
    hp_w2 = din("hp_w2", [NENT, D, D], MM_DT)
    hp_b2 = din("hp_b2", [NENT, D])
    hp_w3 = din("hp_w3", [NENT, D, D], MM_DT)
    hp_b3 = din("hp_b3", [NENT, D])
    local_cW = din("local_cW", [NENT, 2, D, 2 * D], MM_DT)
    local_cb = din("local_cb", [NENT, 2, 2 * D])
    local_vW = din("local_vW", [NENT, 2, D, D], MM_DT)
    local_vb = din("local_vb", [NENT, 2, D])
    global_cW = din("global_cW", [2, D, 2 * D], MM_DT)
    global_cb = din("global_cb", [2, 2 * D])
    global_vW = din("global_vW", [2, D, D], MM_DT)
    global_vb = din("global_vb", [2, D])
    satt_w1 = din("satt_w1", [D, AH], MM_DT)
    satt_w2 = din("satt_w2", [AH, 1], MM_DT)
    band_mask8 = din("band_mask8", [NC, P, L], MM_DT)
    gmat_in = din("gmat_in", [P, 2 * P], MM_DT)
    sel_in = din("sel_in", [NH // 2, NH, P], MM_DT)

    out_a = nc.dram_tensor("out_a", [BPC, L, D], F32, kind="ExternalOutput").ap()
    out_sattw = nc.dram_tensor("out_sattw", [BPC, NENT], F32, kind="ExternalOutput").ap()

    with tile.TileContext(nc) as tc:
        _emit(nc, tc, locals())
    nc.compile()
    return nc


def _autoname(pool):
    orig = pool.tile

    def tile(shape, dtype, **kw):
        if "name" not in kw:
            kw["name"] = kw.get("tag") or "tmp"
        return orig(shape, dtype, **kw)

    pool.tile = tile
    return pool


def _emit(nc, tc, t):
    from contextlib import ExitStack

    ctx = ExitStack()
    const = ctx.enter_context(tc.tile_pool(name="const", bufs=1))
    wpool = ctx.enter_context(tc.tile_pool(name="wpool", bufs=2))
    bpool = ctx.enter_context(tc.tile_pool(name="bpool", bufs=2))
    fpool = ctx.enter_context(tc.tile_pool(name="fpool", bufs=1))
    apool = ctx.enter_context(tc.tile_pool(name="apool", bufs=1))
    spool = ctx.enter_context(tc.tile_pool(name="spool", bufs=1))  # segT
    hpool = ctx.enter_context(tc.tile_pool(name="hpool", bufs=1))  # h1
    kpool = ctx.enter_context(tc.tile_pool(name="kpool", bufs=1))  # mk/mq
    vpool = ctx.enter_context(tc.tile_pool(name="vpool", bufs=1))  # mv_aug
    epool = ctx.enter_context(tc.tile_pool(name="epool", bufs=1))  # exp(scores)
    rpool = ctx.enter_context(tc.tile_pool(name="rpool", bufs=2))  # resid temps
    npool = ctx.enter_context(tc.tile_pool(name="npool", bufs=2))  # nat in/out

    ps_proj = ctx.enter_context(tc.tile_pool(name="ps_proj", bufs=2, space="PSUM"))
    ps_score = ctx.enter_context(tc.tile_pool(name="ps_score", bufs=2, space="PSUM"))
    ps_av = ctx.enter_context(tc.tile_pool(name="ps_av", bufs=2, space="PSUM"))
    ps_bc = ctx.enter_context(tc.tile_pool(name="ps_bc", bufs=2, space="PSUM"))

    for _pl in (const, wpool, bpool, fpool, apool, spool, hpool, kpool, vpool,
                epool, rpool, npool, ps_proj, ps_score, ps_av, ps_bc):
        _autoname(_pl)

    mm = nc.tensor.matmul

    # ---- constants ----
    ones_mat = const.tile([P, P], MM_DT, tag="ones")
    nc.vector.memset(ones_mat.bitcast(F32), 1.0)
    ident = const.tile([P, P], F32, tag="ident")
    from concourse.masks import make_identity
    make_identity(nc, ident)
    if MM_DT != F32:
        ident_mm = const.tile([P, P], MM_DT, tag="identmm")
        nc.vector.tensor_copy(ident_mm, ident)
    else:
        ident_mm = ident

    # gmat[row, 128] = 1 for row in {32, 64}: K=1 gather matmuls slide the
    # free-dim window so head h's denominator lands on psum row h.
    # sel_j [8, 128]: row 2j -> partitions 0:64, row 2j+1 -> 64:128.
    gmat = const.tile([P, 2 * P], MM_DT, tag="gmat")
    nc.sync.dma_start(out=gmat, in_=t["gmat_in"])
    sel = []
    for jj in range(NH // 2):
        s_t = const.tile([NH, P], MM_DT, tag=f"sel{jj}")
        nc.sync.dma_start(out=s_t, in_=t["sel_in"][jj])
        sel.append(s_t)

    band_sb = [const.tile([P, L], MM_DT, tag=f"band{c}") for c in range(NC)]
    for c in range(NC):
        nc.sync.dma_start(out=band_sb[c], in_=t["band_mask8"][c])

    # seg mask -> additive bias per m-partition: (mask-1)*1e9
    negb = const.tile([P, 1], F32, tag="negb")
    nc.vector.memset(negb, NEG)
    mbias = [[const.tile([P, 1], F32, tag=f"mb{b}{c}") for c in range(NC)]
             for b in range(BPC)]
    for b in range(BPC):
        for c in range(NC):
            nc.sync.dma_start(out=mbias[b][c], in_=t["seg_masks"][b, c * P:(c + 1) * P])
            nc.scalar.activation(out=mbias[b][c], in_=mbias[b][c], func=AF.Identity,
                                 scale=-NEG, bias=negb)

    # seT[ki][p, b, n] = se_feats[b, n, 128ki + p]
    seT = [const.tile([P, BPC, NENT], MM_DT, tag=f"seT{ki}") for ki in range(NC)]
    for ki in range(NC):
        ap = bass.AP(tensor=t["se_feats"].tensor, offset=ki * P,
                     ap=[[1, P], [NENT * D, BPC], [D, NENT]])
        nc.sync.dma_start(out=seT[ki], in_=ap)

    # ---- attentive pooling (se_feats only) -> sattwB [128, (b,n)] ----
    sw1 = [const.tile([P, AH], MM_DT, tag=f"sw1{ki}") for ki in range(NC)]
    for ki in range(NC):
        nc.sync.dma_start(out=sw1[ki], in_=t["satt_w1"][ki * P:(ki + 1) * P, :])
    sw2 = [const.tile([P, 1], MM_DT, tag=f"sw2{a}") for a in range(AH // P)]
    for a in range(AH // P):
        nc.sync.dma_start(out=sw2[a], in_=t["satt_w2"][a * P:(a + 1) * P, :])

    th = [const.tile([P, BPC * NENT], MM_DT, tag=f"th{a}") for a in range(AH // P)]
    for a in range(AH // P):
        ps = ps_proj.tile([P, BPC * NENT], F32, tag="proj")
        for ki in range(NC):
            mm(ps, sw1[ki][:, a * P:(a + 1) * P],
               seT[ki].rearrange("p b n -> p (b n)"),
               start=(ki == 0), stop=(ki == NC - 1))
        nc.scalar.activation(out=th[a], in_=ps, func=AF.Tanh)
    ps_alpha = ps_av.tile([1, BPC * NENT], F32, tag="av")
    for a in range(AH // P):
        mm(ps_alpha, sw2[a], th[a], start=(a == 0), stop=(a == AH // P - 1))
    ealpha = const.tile([1, BPC * NENT], F32, tag="ealpha")
    # magnitudes are tiny: skip max-subtraction in these softmaxes
    nc.scalar.activation(out=ealpha, in_=ps_alpha, func=AF.Exp)
    asum = const.tile([1, BPC], F32, tag="asum")
    nc.vector.reduce_sum(out=asum, in_=ealpha.rearrange("o (b n) -> o b n", b=BPC),
                         axis=mybir.AxisListType.X)
    arecip = const.tile([1, BPC], F32, tag="arecip")
    nc.vector.reciprocal(arecip, asum)
    sattw = const.tile([1, BPC * NENT], MM_DT, tag="sattw")
    for b in range(BPC):
        nc.vector.tensor_scalar_mul(sattw[:, b * NENT:(b + 1) * NENT],
                                    ealpha[:, b * NENT:(b + 1) * NENT],
                                    arecip[:, b:b + 1])
    nc.gpsimd.dma_start(out=t["out_sattw"].rearrange("b n -> (b n)")[None, :], in_=sattw)
    ps_sw = ps_bc.tile([P, BPC * NENT], F32, tag="bc")
    mm(ps_sw, ones_mat[0:1, :], sattw, start=True, stop=True)
    sattwB = const.tile([P, BPC * NENT], F32, tag="sattwB")
    nc.scalar.copy(sattwB, ps_sw)

    # ---- h2[n] = relu(se_feats[:, n] @ hp_w2[n] + hp_b2[n]), all (b, n) ----
    h2T = [[const.tile([P, BPC], F32, tag=f"h2{n}{c}") for c in range(NC)]
           for n in range(NENT)]
    for n in range(NENT):
        w2 = [wpool.tile([P, D], MM_DT, tag=f"w1_{ki}", bufs=1) for ki in range(NC)]
        for ki in range(NC):
            nc.sync.dma_start(out=w2[ki], in_=t["hp_w2"][n, ki * P:(ki + 1) * P, :])
        for c in range(NC):
            b2 = bpool.tile([P, 1], F32, tag="b2")
            nc.sync.dma_start(out=b2, in_=t["hp_b2"][n, c * P:(c + 1) * P])
            ps = ps_proj.tile([P, BPC], F32, tag="proj")
            for ki in range(NC):
                mm(ps, w2[ki][:, c * P:(c + 1) * P], seT[ki][:, :, n],
                   start=(ki == 0), stop=(ki == NC - 1))
            nc.scalar.activation(out=h2T[n][c], in_=ps, func=AF.Relu, bias=b2)

    # ---- transpose seg_feats -> segT[b][dc] [128(d), 512(l)] ----
    segT = [[spool.tile([P, L], MM_DT, tag=f"seg{b}{dc}") for dc in range(NC)]
            for b in range(BPC)]
    for b in range(BPC):
        for lc in range(NC):
            nat = npool.tile([P, D], F32, tag="nat", bufs=1)
            nc.sync.dma_start(out=nat, in_=t["seg_feats"][b, lc * P:(lc + 1) * P, :])
            for dc in range(NC):
                pst = ps_score.tile([P, P], F32, tag="score")
                nc.tensor.transpose(pst, nat[:, dc * P:(dc + 1) * P], ident)
                nc.scalar.copy(segT[b][dc][:, lc * P:(lc + 1) * P], pst)

    fT = [[fpool.tile([P, L], MM_DT, tag=f"f{b}{c}") for c in range(NC)]
          for b in range(BPC)]
    aT = [[apool.tile([P, L], MM_DT, tag=f"a{b}{c}") for c in range(NC)]
          for b in range(BPC)]

    def load_biases(src_row, tag):
        """4x [128,1] f32 bias tiles from a length-512 DRAM row."""
        tiles = []
        for c in range(NC):
            bt = bpool.tile([P, 1], F32, tag=f"{tag}{c}")
            nc.sync.dma_start(out=bt, in_=src_row[c * P:(c + 1) * P])
            tiles.append(bt)
        return tiles

    def nl_block(b, xT, cw, vw, cbq, cbv_row, vb, banded):
        """One non-local block, in-place residual update of xT (4 tiles)."""
        mk = [kpool.tile([P, L], MM_DT, tag=f"mk{c}") for c in range(NC)]
        mq = [kpool.tile([P, L], MM_DT, tag=f"mq{c}") for c in range(NC)]
        for c in range(NC):
            ps = ps_proj.tile([P, L], F32, tag="proj")
            for ki in range(NC):
                mm(ps, vw[ki][:, c * P:(c + 1) * P], xT[ki],
                   start=(ki == 0), stop=(ki == NC - 1))
            nc.scalar.activation(out=mk[c], in_=ps, func=AF.Identity, bias=vb[c])
            ps = ps_proj.tile([P, L], F32, tag="proj")
            for ki in range(NC):
                mm(ps, cw[ki][:, c * P:(c + 1) * P], xT[ki],
                   start=(ki == 0), stop=(ki == NC - 1))
            nc.scalar.activation(out=mq[c], in_=ps, func=AF.Identity, bias=cbq[c])
        # mv in natural layout [m, dv], embedded as the stationary operand of
        # the AV matmul so each head's result lands on its own psum partitions
        # (trn2 matmul output always starts at partition 0): even heads use
        # columns 0:64 + ones at 64; odd heads columns 64:128 + ones at 32.
        # Unused columns are zero.
        mv = [vpool.tile([P, NH, P], MM_DT, tag=f"mv{c}") for c in range(NC)]
        for c in range(NC):
            ps = ps_proj.tile([P, L], F32, tag="proj")
            for ki in range(NC):
                mm(ps, xT[ki][:, c * P:(c + 1) * P], cw[ki][:, D:2 * D],
                   start=(ki == 0), stop=False)
            mm(ps, ones_mat[0:1, :], cbv_row, start=False, stop=True)
            psv = ps[:].rearrange("p (h d) -> p h d", h=NH)
            nc.gpsimd.memset(mv[c].bitcast(F32), 0.0)
            nc.vector.tensor_copy(mv[c][:, 0::2, 0:DH], psv[:, 0::2, :])
            nc.vector.tensor_copy(mv[c][:, 1::2, DH:P], psv[:, 1::2, :])
            nc.gpsimd.memset(mv[c][:, 0::2, DH:DH + 1].bitcast(F32), 1.0)
            nc.gpsimd.memset(mv[c][:, 1::2, 32:33].bitcast(F32), 1.0)
        # per-head scores + AV; denominators are gathered onto rows 0..7 of
        # one psum tile so a single reciprocal serves the whole block, and
        # each head's rT slice is copied to a per-pair SBUF tile (its psum
        # bank is then free for the next head).
        stack_ps = ps_bc.tile([P, L], F32, tag="bc")
        rts = []
        for h in range(NH):
            j, base = h // 2, (h % 2) * DH
            eT = []
            for c in range(NC):
                ps = ps_score.tile([P, L], F32, tag="score")
                mm(ps, mq[j][base:base + DH, c * P:(c + 1) * P],
                   mk[j][base:base + DH, :], start=True, stop=not banded)
                if banded:
                    # band mask added on PE: psum += I.T @ mask
                    mm(ps, ident_mm, band_sb[c], start=False, stop=True)
                e = epool.tile([P, L], MM_DT, tag=f"e{c}")
                nc.scalar.activation(out=e, in_=ps, func=AF.Exp,
                                     scale=1.0 / np.sqrt(DH), bias=mbias[b][c])
                eT.append(e)
            # AV: the head-parity column placement in mv makes rT land on
            # the head's own partitions; denominator on a spare aligned row.
            psr = ps_av.tile([P, L], F32, tag="av")
            for c in range(NC):
                mm(psr, mv[c][:, h, :], eT[c], start=(c == 0), stop=(c == NC - 1))
            row = DH if h % 2 == 0 else 32
            if h % 2 == 0:
                rt = rpool.tile([P, L], F32, tag=f"rt{j % 2}")
                rts.append(rt)
            else:
                rt = rts[j]
            nc.vector.tensor_copy(rt[base:base + DH, :], psr[base:base + DH, :])
            dnr = rpool.tile([P, L], MM_DT, tag="dnr")
            nc.scalar.copy(dnr[row:row + 1, :], psr[row:row + 1, :])
            mm(stack_ps, gmat[row:row + 1, P - h:2 * P - h], dnr[row:row + 1, :],
               start=(h == 0), stop=(h == NH - 1))
        rc8 = rpool.tile([NH, L], F32, tag="recip", bufs=1)
        nc.vector.reciprocal_approx_fast(out=rc8, in_=stack_ps[0:NH, :])
        rc8r = rpool.tile([NH, L], MM_DT, tag="recipr", bufs=1)
        nc.scalar.copy(rc8r, rc8)
        for jj in range(NH // 2):
            psb = ps_bc.tile([P, L], F32, tag="bc")
            mm(psb, sel[jj], rc8r, start=True, stop=True)
            rb = rpool.tile([P, L], F32, tag="rb", bufs=1)
            nc.scalar.copy(rb, psb)
            tt = rpool.tile([P, L], F32, tag="t", bufs=1)
            nc.vector.tensor_mul(tt, rts[jj], rb)
            nc.vector.tensor_add(out=xT[jj], in0=xT[jj], in1=tt)

    # ---- main pipeline over entities ----
    for n in range(NENT):
        w1 = [wpool.tile([P, D], MM_DT, tag=f"w1_{ki}", bufs=1) for ki in range(NC)]
        w3 = [wpool.tile([P, D], MM_DT, tag=f"w3_{ki}", bufs=1) for ki in range(NC)]
        for ki in range(NC):
            nc.sync.dma_start(out=w1[ki], in_=t["hp_w1"][n, ki * P:(ki + 1) * P, :])
            nc.sync.dma_start(out=w3[ki], in_=t["hp_w3"][n, ki * P:(ki + 1) * P, :])
        b1 = load_biases(t["hp_b1"][n], "b1")
        b3 = load_biases(t["hp_b3"][n], "b3")
        for b in range(BPC):
            h1 = [hpool.tile([P, L], MM_DT, tag=f"h1{c}") for c in range(NC)]
            for c in range(NC):
                ps = ps_proj.tile([P, L], F32, tag="proj")
                for ki in range(NC):
                    mm(ps, w1[ki][:, c * P:(c + 1) * P], segT[b][ki],
                       start=(ki == 0), stop=(ki == NC - 1))
                nc.scalar.activation(out=h1[c], in_=ps, func=AF.Relu, bias=b1[c])
                nc.vector.tensor_scalar_mul(h1[c], h1[c], h2T[n][c][:, b:b + 1])
            for c in range(NC):
                ps = ps_proj.tile([P, L], F32, tag="proj")
                for ki in range(NC):
                    mm(ps, w3[ki][:, c * P:(c + 1) * P], h1[ki],
                       start=(ki == 0), stop=(ki == NC - 1))
                nc.scalar.activation(out=fT[b][c], in_=ps, func=AF.Relu, bias=b3[c])
        for s in range(2):
            cw = [wpool.tile([P, 2 * D], MM_DT, tag=f"cw{ki}") for ki in range(NC)]
            vw = [wpool.tile([P, D], MM_DT, tag=f"vw{ki}") for ki in range(NC)]
            for ki in range(NC):
                nc.sync.dma_start(out=cw[ki], in_=t["local_cW"][n, s, ki * P:(ki + 1) * P, :])
                nc.sync.dma_start(out=vw[ki], in_=t["local_vW"][n, s, ki * P:(ki + 1) * P, :])
            cbq = load_biases(t["local_cb"][n, s, 0:D], "cbq")
            vb = load_biases(t["local_vb"][n, s], "vb")
            cbv_row = bpool.tile([1, D], MM_DT, tag="cbv")
            nc.gpsimd.dma_start(out=cbv_row, in_=t["local_cb"][n, s, D:2 * D][None, :])
            for b in range(BPC):
                nl_block(b, fT[b], cw, vw, cbq, cbv_row, vb, banded=True)
        # accumulate entity result into aT, weighted by sattw[b, n]
        for b in range(BPC):
            w_ap = sattwB[:, b * NENT + n:b * NENT + n + 1]
            for c in range(NC):
                if n == 0:
                    nc.vector.tensor_scalar_mul(aT[b][c], fT[b][c], w_ap)
                else:
                    wt = rpool.tile([P, L], F32, tag="t", bufs=1)
                    nc.scalar.activation(out=wt, in_=fT[b][c], func=AF.Copy, scale=w_ap)
                    nc.vector.tensor_add(out=aT[b][c], in0=aT[b][c], in1=wt)

    # ---- global blocks ----
    for s in range(2):
        cw = [wpool.tile([P, 2 * D], MM_DT, tag=f"cw{ki}") for ki in range(NC)]
        vw = [wpool.tile([P, D], MM_DT, tag=f"vw{ki}") for ki in range(NC)]
        for ki in range(NC):
            nc.sync.dma_start(out=cw[ki], in_=t["global_cW"][s, ki * P:(ki + 1) * P, :])
            nc.sync.dma_start(out=vw[ki], in_=t["global_vW"][s, ki * P:(ki + 1) * P, :])
        cbq = load_biases(t["global_cb"][s, 0:D], "cbq")
        vb = load_biases(t["global_vb"][s], "vb")
        cbv_row = bpool.tile([1, D], MM_DT, tag="cbv")
        nc.gpsimd.dma_start(out=cbv_row, in_=t["global_cb"][s, D:2 * D][None, :])
        for b in range(BPC):
            nl_block(b, aT[b], cw, vw, cbq, cbv_row, vb, banded=False)

    # ---- transpose back and write out ----
    for b in range(BPC):
        for lc in range(NC):
            out_nat = npool.tile([P, D], F32, tag="on", bufs=1)
            for dc in range(NC):
                pst = ps_score.tile([P, P], MM_DT, tag="score")
                nc.tensor.transpose(pst, aT[b][dc][:, lc * P:(lc + 1) * P], ident_mm)
                nc.scalar.copy(out_nat[:, dc * P:(dc + 1) * P], pst)
            nc.sync.dma_start(out=t["out_a"][b, lc * P:(lc + 1) * P, :], in_=out_nat)

    ctx.close()


_NC_CACHE = None


def kernel(**inputs):
    global _NC_CACHE
    if _NC_CACHE is None:
        _NC_CACHE = _build()
    nc = _NC_CACHE

    band = _band_mask8()
    gmat_np, sel_np = _sel_gmat()
    shared = {k: np.ascontiguousarray(np.asarray(inputs[k], dtype=np.float32))
              for k in ("hp_w1", "hp_b1", "hp_w2", "hp_b2", "hp_w3", "hp_b3",
                        "local_cW", "local_cb", "local_vW", "local_vb",
                        "global_cW", "global_cb", "global_vW", "global_vb",
                        "satt_w1", "satt_w2")}
    seg_feats = np.asarray(inputs["seg_feats"], dtype=np.float32)
    seg_masks = np.asarray(inputs["seg_masks"], dtype=np.float32)
    se_feats = np.asarray(inputs["se_feats"], dtype=np.float32)

    in_maps = []
    for c in range(CORES):
        sl = slice(c * BPC, (c + 1) * BPC)
        m = dict(shared)
        m["seg_feats"] = np.ascontiguousarray(seg_feats[sl])
        m["seg_masks"] = np.ascontiguousarray(seg_masks[sl])
        m["se_feats"] = np.ascontiguousarray(se_feats[sl])
        m["band_mask8"] = band
        m["gmat_in"] = gmat_np
        m["sel_in"] = sel_np
        in_maps.append(m)

    res = bass_utils.run_bass_kernel_spmd(nc, in_maps, core_ids=list(range(CORES)))
    a = np.concatenate([res.results[c]["out_a"] for c in range(CORES)], axis=0)
    sattw = np.concatenate([res.results[c]["out_sattw"] for c in range(CORES)], axis=0)
    return a, sattw


# revision 19
# speedup vs baseline: 1.4569x; 1.1303x over previous
"""Trainium2 Bass kernel for nn_LocalGlobalVideoTextInteractions.

Data-parallel over batch: B=16 across 8 NeuronCores (2 batches/core).
All activations are kept in transposed layout fT[d, l] (feature on the
partition dim) so every projection is a natural PE matmul; attention scores
are computed directly in transposed form sT[m, l], softmax denominators come
from a ones-column appended to the value matrix, and normalization is applied
via a K=1 broadcast matmul + vector ops.

Matmul operand dtype is switchable: float32 (exact, 4 cyc/row) or float32r
(tf32-like, 1 cyc/row at N>=256).
"""
import os
import sys

import numpy as np

for _p in ("/opt/trn_rl_repo", "/root/.axon_site/_ro/trn_rl_repo"):
    if os.path.isdir(_p) and _p not in sys.path:
        sys.path.append(_p)

import concourse.bass as bass
import concourse.tile as tile
from concourse import bacc, mybir
from concourse import bass_utils

F32 = mybir.dt.float32
AF = mybir.ActivationFunctionType

B, L, D = 16, 512, 512
NENT, NH, DH = 3, 8, 64
AH = 256
CORES = 8
BPC = B // CORES  # batches per core
P = 128
NC = D // P  # 4 chunks
KW = 7  # band half-width (ksize 15, dilation 1)
NEG = -1e9
WIN = 144  # banded score window width (>= 128 + 2*KW)
LW0 = [0, 121, 249, 368]  # window start per m-chunk: clamp(128c-7, 0, 512-WIN)
BF16 = mybir.dt.bfloat16

# MM_DT: dtype of every SBUF tensor that feeds the PE array.
MM_DT = mybir.dt.float32r if os.environ.get("KERNEL_F32R", "1") == "1" else F32
BANDED = os.environ.get("KERNEL_BANDED", "0") == "1"  # (banded local path, v2)


def _band_mask8() -> np.ndarray:
    """Additive pre-scale band mask on the score windows, transposed
    orientation: tile c covers m in [128c, 128c+128) on partitions and
    l in [LW0[c], LW0[c]+WIN) on the free dim. 0 in band, -8e9 outside."""
    m = (np.arange(NC)[:, None, None] * P + np.arange(P)[None, :, None])
    l = np.asarray(LW0)[:, None, None] + np.arange(WIN)[None, None, :]
    return np.where(np.abs(l - m) <= KW, 0.0, 8.0 * NEG).astype(np.float32)


def _sel_gmat():
    gmat = np.zeros((P, 2 * P), np.float32)
    gmat[DH, P] = 1.0
    gmat[32, P] = 1.0
    sel = np.zeros((NH // 2, NH, P), np.float32)
    for jj in range(NH // 2):
        sel[jj, 2 * jj, 0:DH] = 1.0
        sel[jj, 2 * jj + 1, DH:P] = 1.0
    return gmat, sel


def _build():
    nc = bacc.Bacc("TRN2", target_bir_lowering=False, debug=False)

    def din(name, shape, dt=F32):
        return nc.dram_tensor(name, shape, dt, kind="ExternalInput").ap()

    seg_feats = din("seg_feats", [BPC, L, D])
    seg_masks = din("seg_masks", [BPC, L])
    se_feats = din("se_feats", [BPC, NENT, D], MM_DT)
    hp_w1 = din("hp_w1", [NENT, D, D], MM_DT)
    hp_b1 = din("hp_b1", [NENT, D])
    hp_w2 = din("hp_w2", [NENT, D, D], MM_DT)
    hp_b2 = din("hp_b2", [NENT, D])
    hp_w3 = din("hp_w3", [NENT, D, D], MM_DT)
    hp_b3 = din("hp_b3", [NENT, D])
    local_cW = din("local_cW", [NENT, 2, D, 2 * D], MM_DT)
    local_cb = din("local_cb", [NENT, 2, 2 * D])
    local_vW = din("local_vW", [NENT, 2, D, D], MM_DT)
    local_vb = din("local_vb", [NENT, 2, D])
    global_cW = din("global_cW", [2, D, 2 * D], MM_DT)
    global_cb = din("global_cb", [2, 2 * D])
    global_vW = din("global_vW", [2, D, D], MM_DT)
    global_vb = din("global_vb", [2, D])
    satt_w1 = din("satt_w1", [D, AH], MM_DT)
    satt_w2 = din("satt_w2", [AH, 1], MM_DT)
    band_mask8 = din("band_mask8", [NC, P, WIN])
    gmat_in = din("gmat_in", [P, 2 * P], MM_DT)
    sel_in = din("sel_in", [NH // 2, NH, P], MM_DT)

    out_a = nc.dram_tensor("out_a", [BPC, L, D], F32, kind="ExternalOutput").ap()
    out_sattw = nc.dram_tensor("out_sattw", [BPC, NENT], F32, kind="ExternalOutput").ap()

    with tile.TileContext(nc) as tc:
        _emit(nc, tc, locals())
    nc.compile()
    return nc


def _autoname(pool):
    orig = pool.tile

    def tile(shape, dtype, **kw):
        if "name" not in kw:
            kw["name"] = kw.get("tag") or "tmp"
        return orig(shape, dtype, **kw)

    pool.tile = tile
    return pool


def _emit(nc, tc, t):
    from contextlib import ExitStack

    ctx = ExitStack()
    const = ctx.enter_context(tc.tile_pool(name="const", bufs=1))
    wpool = ctx.enter_context(tc.tile_pool(name="wpool", bufs=2))
    bpool = ctx.enter_context(tc.tile_pool(name="bpool", bufs=2))
    fpool = ctx.enter_context(tc.tile_pool(name="fpool", bufs=1))
    apool = ctx.enter_context(tc.tile_pool(name="apool", bufs=1))
    spool = ctx.enter_context(tc.tile_pool(name="spool", bufs=1))  # segT
    hpool = ctx.enter_context(tc.tile_pool(name="hpool", bufs=1))  # h1
    kpool = ctx.enter_context(tc.tile_pool(name="kpool", bufs=1))  # mk/mq
    vpool = ctx.enter_context(tc.tile_pool(name="vpool", bufs=1))  # mv_aug
    rpool = ctx.enter_context(tc.tile_pool(name="rpool", bufs=2))  # resid temps
    npool = ctx.enter_context(tc.tile_pool(name="npool", bufs=2))  # nat in/out

    ps_proj = ctx.enter_context(tc.tile_pool(name="ps_proj", bufs=2, space="PSUM"))
    ps_score = ctx.enter_context(tc.tile_pool(name="ps_score", bufs=2, space="PSUM"))
    ps_av = ctx.enter_context(tc.tile_pool(name="ps_av", bufs=2, space="PSUM"))
    ps_bc = ctx.enter_context(tc.tile_pool(name="ps_bc", bufs=2, space="PSUM"))

    for _pl in (const, wpool, bpool, fpool, apool, spool, hpool, kpool, vpool,
                rpool, npool, ps_proj, ps_score, ps_av, ps_bc):
        _autoname(_pl)

    mm = nc.tensor.matmul

    # ---- constants ----
    ones_mat = const.tile([P, P], MM_DT, tag="ones")
    nc.vector.memset(ones_mat.bitcast(F32), 1.0)
    ident = const.tile([P, P], F32, tag="ident")
    from concourse.masks import make_identity
    make_identity(nc, ident)
    if MM_DT != F32:
        ident_mm = const.tile([P, P], MM_DT, tag="identmm")
        nc.vector.tensor_copy(ident_mm, ident)
    else:
        ident_mm = ident

    # gmat[row, 128] = 1 for row in {32, 64}: K=1 gather matmuls slide the
    # free-dim window so head h's denominator lands on psum row h.
    # sel_j [8, 128]: row 2j -> partitions 0:64, row 2j+1 -> 64:128.
    gmat = const.tile([P, 2 * P], MM_DT, tag="gmat")
    nc.sync.dma_start(out=gmat, in_=t["gmat_in"])
    sel = []
    for jj in range(NH // 2):
        s_t = const.tile([NH, P], MM_DT, tag=f"sel{jj}")
        nc.sync.dma_start(out=s_t, in_=t["sel_in"][jj])
        sel.append(s_t)

    band_sb = [const.tile([P, WIN], F32, tag=f"band{c}") for c in range(NC)]
    for c in range(NC):
        nc.sync.dma_start(out=band_sb[c], in_=t["band_mask8"][c])

    # persistent exp(score) tiles: outside-window region stays zero forever
    eTt = [const.tile([P, L], MM_DT, tag=f"e{c}") for c in range(NC)]
    for c in range(NC):
        nc.vector.memset(eTt[c].bitcast(F32), 0.0)

    # seg mask -> additive bias per m-partition: (mask-1)*1e9
    negb = const.tile([P, 1], F32, tag="negb")
    nc.vector.memset(negb, NEG)
    mbias = [[const.tile([P, 1], F32, tag=f"mb{b}{c}") for c in range(NC)]
             for b in range(BPC)]
    for b in range(BPC):
        for c in range(NC):
            nc.sync.dma_start(out=mbias[b][c], in_=t["seg_masks"][b, c * P:(c + 1) * P])
            nc.scalar.activation(out=mbias[b][c], in_=mbias[b][c], func=AF.Identity,
                                 scale=-NEG, bias=negb)

    # seT[ki][p, b, n] = se_feats[b, n, 128ki + p]
    seT = [const.tile([P, BPC, NENT], MM_DT, tag=f"seT{ki}") for ki in range(NC)]
    for ki in range(NC):
        ap = bass.AP(tensor=t["se_feats"].tensor, offset=ki * P,
                     ap=[[1, P], [NENT * D, BPC], [D, NENT]])
        nc.sync.dma_start(out=seT[ki], in_=ap)

    # ---- attentive pooling (se_feats only) -> sattwB [128, (b,n)] ----
    sw1 = [const.tile([P, AH], MM_DT, tag=f"sw1{ki}") for ki in range(NC)]
    for ki in range(NC):
        nc.sync.dma_start(out=sw1[ki], in_=t["satt_w1"][ki * P:(ki + 1) * P, :])
    sw2 = [const.tile([P, 1], MM_DT, tag=f"sw2{a}") for a in range(AH // P)]
    for a in range(AH // P):
        nc.sync.dma_start(out=sw2[a], in_=t["satt_w2"][a * P:(a + 1) * P, :])

    th = [const.tile([P, BPC * NENT], MM_DT, tag=f"th{a}") for a in range(AH // P)]
    for a in range(AH // P):
        ps = ps_proj.tile([P, BPC * NENT], F32, tag="proj")
        for ki in range(NC):
            mm(ps, sw1[ki][:, a * P:(a + 1) * P],
               seT[ki].rearrange("p b n -> p (b n)"),
               start=(ki == 0), stop=(ki == NC - 1))
        nc.scalar.activation(out=th[a], in_=ps, func=AF.Tanh)
    ps_alpha = ps_av.tile([1, BPC * NENT], F32, tag="av")
    for a in range(AH // P):
        mm(ps_alpha, sw2[a], th[a], start=(a == 0), stop=(a == AH // P - 1))
    ealpha = const.tile([1, BPC * NENT], F32, tag="ealpha")
    # magnitudes are tiny: skip max-subtraction in these softmaxes
    nc.scalar.activation(out=ealpha, in_=ps_alpha, func=AF.Exp)
    asum = const.tile([1, BPC], F32, tag="asum")
    nc.vector.reduce_sum(out=asum, in_=ealpha.rearrange("o (b n) -> o b n", b=BPC),
                         axis=mybir.AxisListType.X)
    arecip = const.tile([1, BPC], F32, tag="arecip")
    nc.vector.reciprocal(arecip, asum)
    sattw = const.tile([1, BPC * NENT], MM_DT, tag="sattw")
    for b in range(BPC):
        nc.vector.tensor_scalar_mul(sattw[:, b * NENT:(b + 1) * NENT],
                                    ealpha[:, b * NENT:(b + 1) * NENT],
                                    arecip[:, b:b + 1])
    nc.gpsimd.dma_start(out=t["out_sattw"].rearrange("b n -> (b n)")[None, :], in_=sattw)
    ps_sw = ps_bc.tile([P, BPC * NENT], F32, tag="bc")
    mm(ps_sw, ones_mat[0:1, :], sattw, start=True, stop=True)
    sattwB = const.tile([P, BPC * NENT], F32, tag="sattwB")
    nc.scalar.copy(sattwB, ps_sw)

    # ---- h2[n] = relu(se_feats[:, n] @ hp_w2[n] + hp_b2[n]), all (b, n) ----
    h2T = [[const.tile([P, BPC], F32, tag=f"h2{n}{c}") for c in range(NC)]
           for n in range(NENT)]
    for n in range(NENT):
        w2 = [wpool.tile([P, D], MM_DT, tag=f"w1_{ki}", bufs=1) for ki in range(NC)]
        for ki in range(NC):
            nc.sync.dma_start(out=w2[ki], in_=t["hp_w2"][n, ki * P:(ki + 1) * P, :])
        for c in range(NC):
            b2 = bpool.tile([P, 1], F32, tag="b2")
            nc.sync.dma_start(out=b2, in_=t["hp_b2"][n, c * P:(c + 1) * P])
            ps = ps_proj.tile([P, BPC], F32, tag="proj")
            for ki in range(NC):
                mm(ps, w2[ki][:, c * P:(c + 1) * P], seT[ki][:, :, n],
                   start=(ki == 0), stop=(ki == NC - 1))
            nc.scalar.activation(out=h2T[n][c], in_=ps, func=AF.Relu, bias=b2)

    # ---- transpose seg_feats -> segT[b][dc] [128(d), 512(l)] ----
    segT = [[spool.tile([P, L], MM_DT, tag=f"seg{b}{dc}") for dc in range(NC)]
            for b in range(BPC)]
    for b in range(BPC):
        for lc in range(NC):
            nat = npool.tile([P, D], F32, tag="nat", bufs=1)
            nc.sync.dma_start(out=nat, in_=t["seg_feats"][b, lc * P:(lc + 1) * P, :])
            for dc in range(NC):
                pst = ps_score.tile([P, P], F32, tag="score")
                nc.tensor.transpose(pst, nat[:, dc * P:(dc + 1) * P], ident)
                nc.scalar.copy(segT[b][dc][:, lc * P:(lc + 1) * P], pst)

    fT = [[fpool.tile([P, L], MM_DT, tag=f"f{b}{c}") for c in range(NC)]
          for b in range(BPC)]
    aT = [[apool.tile([P, L], MM_DT, tag=f"a{b}{c}") for c in range(NC)]
          for b in range(BPC)]

    def load_biases(src_row, tag):
        """4x [128,1] f32 bias tiles from a length-512 DRAM row."""
        tiles = []
        for c in range(NC):
            bt = bpool.tile([P, 1], F32, tag=f"{tag}{c}")
            nc.sync.dma_start(out=bt, in_=src_row[c * P:(c + 1) * P])
            tiles.append(bt)
        return tiles

    def nl_block(b, xT, cw, vw, cbq, cbv_row, vb, banded):
        """One non-local block, in-place residual update of xT (4 tiles)."""
        mk = [kpool.tile([P, L], BF16, tag=f"mk{c}") for c in range(NC)]
        mq = [kpool.tile([P, L], BF16, tag=f"mq{c}") for c in range(NC)]
        for c in range(NC):
            ps = ps_proj.tile([P, L], F32, tag="proj")
            for ki in range(NC):
                mm(ps, vw[ki][:, c * P:(c + 1) * P], xT[ki],
                   start=(ki == 0), stop=(ki == NC - 1))
            nc.scalar.activation(out=mk[c], in_=ps, func=AF.Identity, bias=vb[c])
            ps = ps_proj.tile([P, L], F32, tag="proj")
            for ki in range(NC):
                mm(ps, cw[ki][:, c * P:(c + 1) * P], xT[ki],
                   start=(ki == 0), stop=(ki == NC - 1))
            nc.scalar.activation(out=mq[c], in_=ps, func=AF.Identity, bias=cbq[c])
        # mv in natural layout [m, dv], embedded as the stationary operand of
        # the AV matmul so each head's result lands on its own psum partitions
        # (trn2 matmul output always starts at partition 0): even heads use
        # columns 0:64 + ones at 64; odd heads columns 64:128 + ones at 32.
        # Unused columns are zero.
        mv = [vpool.tile([P, NH, P], MM_DT, tag=f"mv{c}") for c in range(NC)]
        for c in range(NC):
            ps = ps_proj.tile([P, L], F32, tag="proj")
            for ki in range(NC):
                mm(ps, xT[ki][:, c * P:(c + 1) * P], cw[ki][:, D:2 * D],
                   start=(ki == 0), stop=False)
            mm(ps, ones_mat[0:1, :], cbv_row, start=False, stop=True)
            psv = ps[:].rearrange("p (h d) -> p h d", h=NH)
            nc.gpsimd.memset(mv[c].bitcast(F32), 0.0)
            nc.vector.tensor_copy(mv[c][:, 0::2, 0:DH], psv[:, 0::2, :])
            nc.vector.tensor_copy(mv[c][:, 1::2, DH:P], psv[:, 1::2, :])
            nc.gpsimd.memset(mv[c][:, 0::2, DH:DH + 1].bitcast(F32), 1.0)
            nc.gpsimd.memset(mv[c][:, 1::2, 32:33].bitcast(F32), 1.0)
        # per-head scores + AV; denominators are gathered onto rows 0..7 of
        # one psum tile so a single reciprocal serves the whole block, and
        # each head's rT slice is copied to a per-pair SBUF tile (its psum
        # bank is then free for the next head).
        stack_ps = ps_bc.tile([P, L], F32, tag="bc")
        rts = []
        for h in range(NH):
            j, base = h // 2, (h % 2) * DH
            for c in range(NC):
                if banded:
                    ps = ps_score.tile([P, WIN], F32, tag="score")
                    mm(ps, mq[j][base:base + DH, c * P:(c + 1) * P],
                       mk[j][base:base + DH, LW0[c]:LW0[c] + WIN],
                       start=True, stop=True)
                    nc.vector.tensor_add(out=ps, in0=ps, in1=band_sb[c])
                    nc.scalar.activation(out=eTt[c][:, LW0[c]:LW0[c] + WIN],
                                         in_=ps, func=AF.Exp,
                                         scale=1.0 / np.sqrt(DH), bias=mbias[b][c])
                else:
                    ps = ps_score.tile([P, L], F32, tag="score")
                    mm(ps, mq[j][base:base + DH, c * P:(c + 1) * P],
                       mk[j][base:base + DH, :], start=True, stop=True)
                    nc.scalar.activation(out=eTt[c], in_=ps, func=AF.Exp,
                                         scale=1.0 / np.sqrt(DH), bias=mbias[b][c])
            # AV: the head-parity column placement in mv makes rT land on
            # the head's own partitions; denominator on a spare aligned row.
            psr = ps_av.tile([P, L], F32, tag="av")
            for c in range(NC):
                mm(psr, mv[c][:, h, :], eTt[c], start=(c == 0), stop=(c == NC - 1))
            row = DH if h % 2 == 0 else 32
            if h % 2 == 0:
                rt = rpool.tile([P, L], F32, tag=f"rt{j % 2}")
                rts.append(rt)
            else:
                rt = rts[j]
            nc.vector.tensor_copy(rt[base:base + DH, :], psr[base:base + DH, :])
            dnr = rpool.tile([P, L], MM_DT, tag="dnr")
            nc.scalar.copy(dnr[row:row + 1, :], psr[row:row + 1, :])
            mm(stack_ps, gmat[row:row + 1, P - h:2 * P - h], dnr[row:row + 1, :],
               start=(h == 0), stop=(h == NH - 1))
        rc8 = rpool.tile([NH, L], F32, tag="recip", bufs=1)
        nc.vector.reciprocal_approx_fast(out=rc8, in_=stack_ps[0:NH, :])
        rc8r = rpool.tile([NH, L], MM_DT, tag="recipr", bufs=1)
        nc.scalar.copy(rc8r, rc8)
        for jj in range(NH // 2):
            psb = ps_bc.tile([P, L], F32, tag="bc")
            mm(psb, sel[jj], rc8r, start=True, stop=True)
            rb = rpool.tile([P, L], F32, tag="rb", bufs=1)
            nc.scalar.copy(rb, psb)
            tt = rpool.tile([P, L], F32, tag="t", bufs=1)
            nc.vector.tensor_mul(tt, rts[jj], rb)
            nc.vector.tensor_add(out=xT[jj], in0=xT[jj], in1=tt)

    # ---- main pipeline over entities ----
    for n in range(NENT):
        w1 = [wpool.tile([P, D], MM_DT, tag=f"w1_{ki}", bufs=1) for ki in range(NC)]
        w3 = [wpool.tile([P, D], MM_DT, tag=f"w3_{ki}", bufs=1) for ki in range(NC)]
        for ki in range(NC):
            nc.sync.dma_start(out=w1[ki], in_=t["hp_w1"][n, ki * P:(ki + 1) * P, :])
            nc.sync.dma_start(out=w3[ki], in_=t["hp_w3"][n, ki * P:(ki + 1) * P, :])
        b1 = load_biases(t["hp_b1"][n], "b1")
        b3 = load_biases(t["hp_b3"][n], "b3")
        for b in range(BPC):
            h1 = [hpool.tile([P, L], MM_DT, tag=f"h1{c}") for c in range(NC)]
            for c in range(NC):
                ps = ps_proj.tile([P, L], F32, tag="proj")
                for ki in range(NC):
                    mm(ps, w1[ki][:, c * P:(c + 1) * P], segT[b][ki],
                       start=(ki == 0), stop=(ki == NC - 1))
                nc.scalar.activation(out=h1[c], in_=ps, func=AF.Relu, bias=b1[c])
                nc.vector.tensor_scalar_mul(h1[c], h1[c], h2T[n][c][:, b:b + 1])
            for c in range(NC):
                ps = ps_proj.tile([P, L], F32, tag="proj")
                for ki in range(NC):
                    mm(ps, w3[ki][:, c * P:(c + 1) * P], h1[ki],
                       start=(ki == 0), stop=(ki == NC - 1))
                nc.scalar.activation(out=fT[b][c], in_=ps, func=AF.Relu, bias=b3[c])
        for s in range(2):
            cw = [wpool.tile([P, 2 * D], MM_DT, tag=f"cw{ki}") for ki in range(NC)]
            vw = [wpool.tile([P, D], MM_DT, tag=f"vw{ki}") for ki in range(NC)]
            for ki in range(NC):
                nc.sync.dma_start(out=cw[ki], in_=t["local_cW"][n, s, ki * P:(ki + 1) * P, :])
                nc.sync.dma_start(out=vw[ki], in_=t["local_vW"][n, s, ki * P:(ki + 1) * P, :])
            cbq = load_biases(t["local_cb"][n, s, 0:D], "cbq")
            vb = load_biases(t["local_vb"][n, s], "vb")
            cbv_row = bpool.tile([1, D], MM_DT, tag="cbv")
            nc.gpsimd.dma_start(out=cbv_row, in_=t["local_cb"][n, s, D:2 * D][None, :])
            for b in range(BPC):
                nl_block(b, fT[b], cw, vw, cbq, cbv_row, vb, banded=True)
        # accumulate entity result into aT, weighted by sattw[b, n]
        for b in range(BPC):
            w_ap = sattwB[:, b * NENT + n:b * NENT + n + 1]
            for c in range(NC):
                if n == 0:
                    nc.vector.tensor_scalar_mul(aT[b][c], fT[b][c], w_ap)
                else:
                    wt = rpool.tile([P, L], F32, tag="t", bufs=1)
                    nc.scalar.activation(out=wt, in_=fT[b][c], func=AF.Copy, scale=w_ap)
                    nc.vector.tensor_add(out=aT[b][c], in0=aT[b][c], in1=wt)

    # ---- global blocks ----
    for s in range(2):
        cw = [wpool.tile([P, 2 * D], MM_DT, tag=f"cw{ki}") for ki in range(NC)]
        vw = [wpool.tile([P, D], MM_DT, tag=f"vw{ki}") for ki in range(NC)]
        for ki in range(NC):
            nc.sync.dma_start(out=cw[ki], in_=t["global_cW"][s, ki * P:(ki + 1) * P, :])
            nc.sync.dma_start(out=vw[ki], in_=t["global_vW"][s, ki * P:(ki + 1) * P, :])
        cbq = load_biases(t["global_cb"][s, 0:D], "cbq")
        vb = load_biases(t["global_vb"][s], "vb")
        cbv_row = bpool.tile([1, D], MM_DT, tag="cbv")
        nc.gpsimd.dma_start(out=cbv_row, in_=t["global_cb"][s, D:2 * D][None, :])
        for b in range(BPC):
            nl_block(b, aT[b], cw, vw, cbq, cbv_row, vb, banded=False)

    # ---- transpose back and write out ----
    for b in range(BPC):
        for lc in range(NC):
            out_nat = npool.tile([P, D], F32, tag="on", bufs=1)
            for dc in range(NC):
                pst = ps_score.tile([P, P], MM_DT, tag="score")
                nc.tensor.transpose(pst, aT[b][dc][:, lc * P:(lc + 1) * P], ident_mm)
                nc.scalar.copy(out_nat[:, dc * P:(dc + 1) * P], pst)
            nc.sync.dma_start(out=t["out_a"][b, lc * P:(lc + 1) * P, :], in_=out_nat)

    ctx.close()


_NC_CACHE = None


def kernel(**inputs):
    global _NC_CACHE
    if _NC_CACHE is None:
        _NC_CACHE = _build()
    nc = _NC_CACHE

    band = _band_mask8()
    gmat_np, sel_np = _sel_gmat()
    shared = {k: np.ascontiguousarray(np.asarray(inputs[k], dtype=np.float32))
              for k in ("hp_w1", "hp_b1", "hp_w2", "hp_b2", "hp_w3", "hp_b3",
                        "local_cW", "local_cb", "local_vW", "local_vb",
                        "global_cW", "global_cb", "global_vW", "global_vb",
                        "satt_w1", "satt_w2")}
    seg_feats = np.asarray(inputs["seg_feats"], dtype=np.float32)
    seg_masks = np.asarray(inputs["seg_masks"], dtype=np.float32)
    se_feats = np.asarray(inputs["se_feats"], dtype=np.float32)

    in_maps = []
    for c in range(CORES):
        sl = slice(c * BPC, (c + 1) * BPC)
        m = dict(shared)
        m["seg_feats"] = np.ascontiguousarray(seg_feats[sl])
        m["seg_masks"] = np.ascontiguousarray(seg_masks[sl])
        m["se_feats"] = np.ascontiguousarray(se_feats[sl])
        m["band_mask8"] = band
        m["gmat_in"] = gmat_np
        m["sel_in"] = sel_np
        in_maps.append(m)

    res = bass_utils.run_bass_kernel_spmd(nc, in_maps, core_ids=list(range(CORES)))
    a = np.concatenate([res.results[c]["out_a"] for c in range(CORES)], axis=0)
    sattw = np.concatenate([res.results[c]["out_sattw"] for c in range(CORES)], axis=0)
    return a, sattw


# revision 21
# speedup vs baseline: 1.5491x; 1.0633x over previous
"""Trainium2 Bass kernel for nn_LocalGlobalVideoTextInteractions.

Data-parallel over batch: B=16 across 8 NeuronCores (2 batches/core).
All activations are kept in transposed layout fT[d, l] (feature on the
partition dim) so every projection is a natural PE matmul; attention scores
are computed directly in transposed form sT[m, l], softmax denominators come
from a ones-column appended to the value matrix, and normalization is applied
via a K=1 broadcast matmul + vector ops.

Matmul operand dtype is switchable: float32 (exact, 4 cyc/row) or float32r
(tf32-like, 1 cyc/row at N>=256).
"""
import os
import sys

import numpy as np

for _p in ("/opt/trn_rl_repo", "/root/.axon_site/_ro/trn_rl_repo"):
    if os.path.isdir(_p) and _p not in sys.path:
        sys.path.append(_p)

import concourse.bass as bass
import concourse.tile as tile
from concourse import bacc, mybir
from concourse import bass_utils

F32 = mybir.dt.float32
AF = mybir.ActivationFunctionType

B, L, D = 16, 512, 512
NENT, NH, DH = 3, 8, 64
AH = 256
CORES = 8
BPC = B // CORES  # batches per core
P = 128
NC = D // P  # 4 chunks
KW = 7  # band half-width (ksize 15, dilation 1)
NEG = -1e9
WIN = 144  # banded score window width (>= 128 + 2*KW)
LW0 = [0, 121, 249, 368]  # window start per m-chunk: clamp(128c-7, 0, 512-WIN)
BF16 = mybir.dt.bfloat16

# MM_DT: dtype of every SBUF tensor that feeds the PE array.
MM_DT = mybir.dt.float32r if os.environ.get("KERNEL_F32R", "1") == "1" else F32
BANDED = os.environ.get("KERNEL_BANDED", "0") == "1"  # (banded local path, v2)


def _band_mask8() -> np.ndarray:
    """Additive pre-scale band mask on the score windows, transposed
    orientation: tile c covers m in [128c, 128c+128) on partitions and
    l in [LW0[c], LW0[c]+WIN) on the free dim. 0 in band, -8e9 outside."""
    m = (np.arange(NC)[:, None, None] * P + np.arange(P)[None, :, None])
    l = np.asarray(LW0)[:, None, None] + np.arange(WIN)[None, None, :]
    return np.where(np.abs(l - m) <= KW, 0.0, 8.0 * NEG).astype(np.float32)


def _sel_gmat():
    gmat = np.zeros((P, 2 * P), np.float32)
    gmat[DH, P] = 1.0
    gmat[32, P] = 1.0
    sel = np.zeros((NH // 2, NH, P), np.float32)
    for jj in range(NH // 2):
        sel[jj, 2 * jj, 0:DH] = 1.0
        sel[jj, 2 * jj + 1, DH:P] = 1.0
    return gmat, sel


def _build():
    nc = bacc.Bacc("TRN2", target_bir_lowering=False, debug=False)

    def din(name, shape, dt=F32):
        return nc.dram_tensor(name, shape, dt, kind="ExternalInput").ap()

    seg_feats = din("seg_feats", [BPC, L, D])
    seg_masks = din("seg_masks", [BPC, L])
    se_feats = din("se_feats", [BPC, NENT, D], MM_DT)
    hp_w1 = din("hp_w1", [NENT, D, D], MM_DT)
    hp_b1 = din("hp_b1", [NENT, D])
    hp_w2 = din("hp_w2", [NENT, D, D], MM_DT)
    hp_b2 = din("hp_b2", [NENT, D])
    hp_w3 = din("hp_w3", [NENT, D, D], MM_DT)
    hp_b3 = din("hp_b3", [NENT, D])
    local_cW = din("local_cW", [NENT, 2, D, 2 * D], MM_DT)
    local_cb = din("local_cb", [NENT, 2, 2 * D])
    local_vW = din("local_vW", [NENT, 2, D, D], MM_DT)
    local_vb = din("local_vb", [NENT, 2, D])
    global_cW = din("global_cW", [2, D, 2 * D], MM_DT)
    global_cb = din("global_cb", [2, 2 * D])
    global_vW = din("global_vW", [2, D, D], MM_DT)
    global_vb = din("global_vb", [2, D])
    satt_w1 = din("satt_w1", [D, AH], MM_DT)
    satt_w2 = din("satt_w2", [AH, 1], MM_DT)
    band_mask8 = din("band_mask8", [NC, P, WIN])
    gmat_in = din("gmat_in", [P, 2 * P], MM_DT)
    sel_in = din("sel_in", [NH // 2, NH, P], MM_DT)

    out_a = nc.dram_tensor("out_a", [BPC, L, D], F32, kind="ExternalOutput").ap()
    out_sattw = nc.dram_tensor("out_sattw", [BPC, NENT], F32, kind="ExternalOutput").ap()

    with tile.TileContext(nc) as tc:
        _emit(nc, tc, locals())
    nc.compile()
    return nc


def _autoname(pool):
    orig = pool.tile

    def tile(shape, dtype, **kw):
        if "name" not in kw:
            kw["name"] = kw.get("tag") or "tmp"
        return orig(shape, dtype, **kw)

    pool.tile = tile
    return pool


def _emit(nc, tc, t):
    from contextlib import ExitStack

    ctx = ExitStack()
    const = ctx.enter_context(tc.tile_pool(name="const", bufs=1))
    wpool = ctx.enter_context(tc.tile_pool(name="wpool", bufs=2))
    bpool = ctx.enter_context(tc.tile_pool(name="bpool", bufs=2))
    fpool = ctx.enter_context(tc.tile_pool(name="fpool", bufs=1))
    apool = ctx.enter_context(tc.tile_pool(name="apool", bufs=1))
    spool = ctx.enter_context(tc.tile_pool(name="spool", bufs=1))  # segT
    hpool = ctx.enter_context(tc.tile_pool(name="hpool", bufs=1))  # h1
    kpool = ctx.enter_context(tc.tile_pool(name="kpool", bufs=2))  # mk/mq
    vpool = ctx.enter_context(tc.tile_pool(name="vpool", bufs=1))  # mv_aug
    rpool = ctx.enter_context(tc.tile_pool(name="rpool", bufs=2))  # resid temps
    npool = ctx.enter_context(tc.tile_pool(name="npool", bufs=2))  # nat in/out

    ps_proj = ctx.enter_context(tc.tile_pool(name="ps_proj", bufs=2, space="PSUM"))
    ps_score = ctx.enter_context(tc.tile_pool(name="ps_score", bufs=2, space="PSUM"))
    ps_av = ctx.enter_context(tc.tile_pool(name="ps_av", bufs=2, space="PSUM"))
    ps_bc = ctx.enter_context(tc.tile_pool(name="ps_bc", bufs=2, space="PSUM"))

    for _pl in (const, wpool, bpool, fpool, apool, spool, hpool, kpool, vpool,
                rpool, npool, ps_proj, ps_score, ps_av, ps_bc):
        _autoname(_pl)

    mm = nc.tensor.matmul

    # ---- constants ----
    ones_mat = const.tile([P, P], MM_DT, tag="ones")
    nc.vector.memset(ones_mat.bitcast(F32), 1.0)
    ident = const.tile([P, P], F32, tag="ident")
    from concourse.masks import make_identity
    make_identity(nc, ident)
    if MM_DT != F32:
        ident_mm = const.tile([P, P], MM_DT, tag="identmm")
        nc.vector.tensor_copy(ident_mm, ident)
    else:
        ident_mm = ident

    # gmat[row, 128] = 1 for row in {32, 64}: K=1 gather matmuls slide the
    # free-dim window so head h's denominator lands on psum row h.
    # sel_j [8, 128]: row 2j -> partitions 0:64, row 2j+1 -> 64:128.
    gmat = const.tile([P, 2 * P], MM_DT, tag="gmat")
    nc.sync.dma_start(out=gmat, in_=t["gmat_in"])
    sel = []
    for jj in range(NH // 2):
        s_t = const.tile([NH, P], MM_DT, tag=f"sel{jj}")
        nc.sync.dma_start(out=s_t, in_=t["sel_in"][jj])
        sel.append(s_t)

    band_sb = [const.tile([P, WIN], F32, tag=f"band{c}") for c in range(NC)]
    for c in range(NC):
        nc.sync.dma_start(out=band_sb[c], in_=t["band_mask8"][c])

    # persistent exp(score) tiles, double-buffered by head parity:
    # outside-window region stays zero forever
    eTt = [[const.tile([P, L], MM_DT, tag=f"e{c}_{p}") for p in range(2)]
           for c in range(NC)]
    for c in range(NC):
        for p in range(2):
            nc.vector.memset(eTt[c][p].bitcast(F32), 0.0)

    # seg mask -> additive bias per m-partition: (mask-1)*1e9
    negb = const.tile([P, 1], F32, tag="negb")
    nc.vector.memset(negb, NEG)
    mbias = [[const.tile([P, 1], F32, tag=f"mb{b}{c}") for c in range(NC)]
             for b in range(BPC)]
    for b in range(BPC):
        for c in range(NC):
            nc.sync.dma_start(out=mbias[b][c], in_=t["seg_masks"][b, c * P:(c + 1) * P])
            nc.scalar.activation(out=mbias[b][c], in_=mbias[b][c], func=AF.Identity,
                                 scale=-NEG, bias=negb)

    # seT[ki][p, b, n] = se_feats[b, n, 128ki + p]
    seT = [const.tile([P, BPC, NENT], MM_DT, tag=f"seT{ki}") for ki in range(NC)]
    for ki in range(NC):
        ap = bass.AP(tensor=t["se_feats"].tensor, offset=ki * P,
                     ap=[[1, P], [NENT * D, BPC], [D, NENT]])
        nc.sync.dma_start(out=seT[ki], in_=ap)

    # ---- attentive pooling (se_feats only) -> sattwB [128, (b,n)] ----
    sw1 = [const.tile([P, AH], MM_DT, tag=f"sw1{ki}") for ki in range(NC)]
    for ki in range(NC):
        nc.sync.dma_start(out=sw1[ki], in_=t["satt_w1"][ki * P:(ki + 1) * P, :])
    sw2 = [const.tile([P, 1], MM_DT, tag=f"sw2{a}") for a in range(AH // P)]
    for a in range(AH // P):
        nc.sync.dma_start(out=sw2[a], in_=t["satt_w2"][a * P:(a + 1) * P, :])

    th = [const.tile([P, BPC * NENT], MM_DT, tag=f"th{a}") for a in range(AH // P)]
    for a in range(AH // P):
        ps = ps_proj.tile([P, BPC * NENT], F32, tag="proj")
        for ki in range(NC):
            mm(ps, sw1[ki][:, a * P:(a + 1) * P],
               seT[ki].rearrange("p b n -> p (b n)"),
               start=(ki == 0), stop=(ki == NC - 1))
        nc.scalar.activation(out=th[a], in_=ps, func=AF.Tanh)
    ps_alpha = ps_av.tile([1, BPC * NENT], F32, tag="av")
    for a in range(AH // P):
        mm(ps_alpha, sw2[a], th[a], start=(a == 0), stop=(a == AH // P - 1))
    ealpha = const.tile([1, BPC * NENT], F32, tag="ealpha")
    # magnitudes are tiny: skip max-subtraction in these softmaxes
    nc.scalar.activation(out=ealpha, in_=ps_alpha, func=AF.Exp)
    asum = const.tile([1, BPC], F32, tag="asum")
    nc.vector.reduce_sum(out=asum, in_=ealpha.rearrange("o (b n) -> o b n", b=BPC),
                         axis=mybir.AxisListType.X)
    arecip = const.tile([1, BPC], F32, tag="arecip")
    nc.vector.reciprocal(arecip, asum)
    sattw = const.tile([1, BPC * NENT], MM_DT, tag="sattw")
    for b in range(BPC):
        nc.vector.tensor_scalar_mul(sattw[:, b * NENT:(b + 1) * NENT],
                                    ealpha[:, b * NENT:(b + 1) * NENT],
                                    arecip[:, b:b + 1])
    nc.gpsimd.dma_start(out=t["out_sattw"].rearrange("b n -> (b n)")[None, :], in_=sattw)
    ps_sw = ps_bc.tile([P, BPC * NENT], F32, tag="bc")
    mm(ps_sw, ones_mat[0:1, :], sattw, start=True, stop=True)
    sattwB = const.tile([P, BPC * NENT], F32, tag="sattwB")
    nc.scalar.copy(sattwB, ps_sw)

    # ---- h2[n] = relu(se_feats[:, n] @ hp_w2[n] + hp_b2[n]), all (b, n) ----
    h2T = [[const.tile([P, BPC], F32, tag=f"h2{n}{c}") for c in range(NC)]
           for n in range(NENT)]
    for n in range(NENT):
        w2 = [wpool.tile([P, D], MM_DT, tag=f"w1_{ki}", bufs=1) for ki in range(NC)]
        for ki in range(NC):
            nc.sync.dma_start(out=w2[ki], in_=t["hp_w2"][n, ki * P:(ki + 1) * P, :])
        for c in range(NC):
            b2 = bpool.tile([P, 1], F32, tag="b2")
            nc.sync.dma_start(out=b2, in_=t["hp_b2"][n, c * P:(c + 1) * P])
            ps = ps_proj.tile([P, BPC], F32, tag="proj")
            for ki in range(NC):
                mm(ps, w2[ki][:, c * P:(c + 1) * P], seT[ki][:, :, n],
                   start=(ki == 0), stop=(ki == NC - 1))
            nc.scalar.activation(out=h2T[n][c], in_=ps, func=AF.Relu, bias=b2)

    # ---- transpose seg_feats -> segT[b][dc] [128(d), 512(l)] ----
    segT = [[spool.tile([P, L], MM_DT, tag=f"seg{b}{dc}") for dc in range(NC)]
            for b in range(BPC)]
    for b in range(BPC):
        for lc in range(NC):
            nat = npool.tile([P, D], F32, tag="nat", bufs=1)
            nc.sync.dma_start(out=nat, in_=t["seg_feats"][b, lc * P:(lc + 1) * P, :])
            for dc in range(NC):
                pst = ps_score.tile([P, P], F32, tag="score")
                nc.tensor.transpose(pst, nat[:, dc * P:(dc + 1) * P], ident)
                nc.scalar.copy(segT[b][dc][:, lc * P:(lc + 1) * P], pst)

    fT = [[fpool.tile([P, L], MM_DT, tag=f"f{b}{c}") for c in range(NC)]
          for b in range(BPC)]
    aT = [[apool.tile([P, L], MM_DT, tag=f"a{b}{c}") for c in range(NC)]
          for b in range(BPC)]

    def load_biases(src_row, tag):
        """4x [128,1] f32 bias tiles from a length-512 DRAM row."""
        tiles = []
        for c in range(NC):
            bt = bpool.tile([P, 1], F32, tag=f"{tag}{c}")
            nc.sync.dma_start(out=bt, in_=src_row[c * P:(c + 1) * P])
            tiles.append(bt)
        return tiles

    def nl_block(b, xT, cw, vw, cbq, cbv_row, vb, banded):
        """One non-local block, in-place residual update of xT (4 tiles)."""
        mk = [kpool.tile([P, L], BF16, tag=f"mk{c}") for c in range(NC)]
        mq = [kpool.tile([P, L], BF16, tag=f"mq{c}") for c in range(NC)]
        for c in range(NC):
            ps = ps_proj.tile([P, L], F32, tag="proj")
            for ki in range(NC):
                mm(ps, vw[ki][:, c * P:(c + 1) * P], xT[ki],
                   start=(ki == 0), stop=(ki == NC - 1))
            nc.scalar.activation(out=mk[c], in_=ps, func=AF.Identity, bias=vb[c])
            ps = ps_proj.tile([P, L], F32, tag="proj")
            for ki in range(NC):
                mm(ps, cw[ki][:, c * P:(c + 1) * P], xT[ki],
                   start=(ki == 0), stop=(ki == NC - 1))
            nc.scalar.activation(out=mq[c], in_=ps, func=AF.Identity, bias=cbq[c])
        # mv in natural layout [m, dv], embedded as the stationary operand of
        # the AV matmul so each head's result lands on its own psum partitions
        # (trn2 matmul output always starts at partition 0): even heads use
        # columns 0:64 + ones at 64; odd heads columns 64:128 + ones at 32.
        # Unused columns are zero.
        mv = [vpool.tile([P, NH, P], MM_DT, tag=f"mv{c}") for c in range(NC)]
        for c in range(NC):
            ps = ps_proj.tile([P, L], F32, tag="proj")
            for ki in range(NC):
                mm(ps, xT[ki][:, c * P:(c + 1) * P], cw[ki][:, D:2 * D],
                   start=(ki == 0), stop=False)
            mm(ps, ones_mat[0:1, :], cbv_row, start=False, stop=True)
            psv = ps[:].rearrange("p (h d) -> p h d", h=NH)
            nc.gpsimd.memset(mv[c].bitcast(F32), 0.0)
            nc.vector.tensor_copy(mv[c][:, 0::2, 0:DH], psv[:, 0::2, :])
            nc.vector.tensor_copy(mv[c][:, 1::2, DH:P], psv[:, 1::2, :])
            nc.gpsimd.memset(mv[c][:, 0::2, DH:DH + 1].bitcast(F32), 1.0)
            nc.gpsimd.memset(mv[c][:, 1::2, 32:33].bitcast(F32), 1.0)
        # per-head scores + AV; denominators are gathered onto rows 0..7 of
        # one psum tile so a single reciprocal serves the whole block, and
        # each head's rT slice is copied to a per-pair SBUF tile (its psum
        # bank is then free for the next head).
        stack_ps = ps_bc.tile([P, L], F32, tag="bc")
        rts = []
        for h in range(NH):
            j, base = h // 2, (h % 2) * DH
            for c in range(NC):
                if banded:
                    ps = ps_score.tile([P, WIN], F32, tag="score")
                    mm(ps, mq[j][base:base + DH, c * P:(c + 1) * P],
                       mk[j][base:base + DH, LW0[c]:LW0[c] + WIN],
                       start=True, stop=True)
                    nc.vector.tensor_add(out=ps, in0=ps, in1=band_sb[c])
                    nc.scalar.activation(out=eTt[c][h % 2][:, LW0[c]:LW0[c] + WIN],
                                         in_=ps, func=AF.Exp,
                                         scale=1.0 / np.sqrt(DH), bias=mbias[b][c])
                else:
                    ps = ps_score.tile([P, L], F32, tag="score")
                    mm(ps, mq[j][base:base + DH, c * P:(c + 1) * P],
                       mk[j][base:base + DH, :], start=True, stop=True)
                    nc.scalar.activation(out=eTt[c][h % 2], in_=ps, func=AF.Exp,
                                         scale=1.0 / np.sqrt(DH), bias=mbias[b][c])
            # AV: the head-parity column placement in mv makes rT land on
            # the head's own partitions; denominator on a spare aligned row.
            psr = ps_av.tile([P, L], F32, tag="av")
            for c in range(NC):
                mm(psr, mv[c][:, h, :], eTt[c][h % 2], start=(c == 0), stop=(c == NC - 1))
            row = DH if h % 2 == 0 else 32
            if h % 2 == 0:
                rt = rpool.tile([P, L], F32, tag=f"rt{j % 2}")
                rts.append(rt)
            else:
                rt = rts[j]
            nc.vector.tensor_copy(rt[base:base + DH, :], psr[base:base + DH, :])
            dnr = rpool.tile([P, L], MM_DT, tag="dnr", bufs=1)
            nc.scalar.copy(dnr[row:row + 1, :], psr[row:row + 1, :])
            mm(stack_ps, gmat[row:row + 1, P - h:2 * P - h], dnr[row:row + 1, :],
               start=(h == 0), stop=(h == NH - 1))
        rc8 = rpool.tile([NH, L], F32, tag="recip", bufs=1)
        nc.vector.reciprocal_approx_fast(out=rc8, in_=stack_ps[0:NH, :])
        rc8r = rpool.tile([NH, L], MM_DT, tag="recipr", bufs=1)
        nc.scalar.copy(rc8r, rc8)
        for jj in range(NH // 2):
            psb = ps_bc.tile([P, L], F32, tag="bc")
            mm(psb, sel[jj], rc8r, start=True, stop=True)
            rb = rpool.tile([P, L], F32, tag="rb", bufs=1)
            nc.scalar.copy(rb, psb)
            tt = rpool.tile([P, L], F32, tag="t", bufs=1)
            nc.vector.tensor_mul(tt, rts[jj], rb)
            nc.vector.tensor_add(out=xT[jj], in0=xT[jj], in1=tt)

    # ---- main pipeline over entities ----
    for n in range(NENT):
        w1 = [wpool.tile([P, D], MM_DT, tag=f"w1_{ki}", bufs=1) for ki in range(NC)]
        w3 = [wpool.tile([P, D], MM_DT, tag=f"w3_{ki}", bufs=1) for ki in range(NC)]
        for ki in range(NC):
            nc.sync.dma_start(out=w1[ki], in_=t["hp_w1"][n, ki * P:(ki + 1) * P, :])
            nc.sync.dma_start(out=w3[ki], in_=t["hp_w3"][n, ki * P:(ki + 1) * P, :])
        b1 = load_biases(t["hp_b1"][n], "b1")
        b3 = load_biases(t["hp_b3"][n], "b3")
        for b in range(BPC):
            h1 = [hpool.tile([P, L], MM_DT, tag=f"h1{c}") for c in range(NC)]
            for c in range(NC):
                ps = ps_proj.tile([P, L], F32, tag="proj")
                for ki in range(NC):
                    mm(ps, w1[ki][:, c * P:(c + 1) * P], segT[b][ki],
                       start=(ki == 0), stop=(ki == NC - 1))
                nc.scalar.activation(out=h1[c], in_=ps, func=AF.Relu, bias=b1[c])
                nc.vector.tensor_scalar_mul(h1[c], h1[c], h2T[n][c][:, b:b + 1])
            for c in range(NC):
                ps = ps_proj.tile([P, L], F32, tag="proj")
                for ki in range(NC):
                    mm(ps, w3[ki][:, c * P:(c + 1) * P], h1[ki],
                       start=(ki == 0), stop=(ki == NC - 1))
                nc.scalar.activation(out=fT[b][c], in_=ps, func=AF.Relu, bias=b3[c])
        for s in range(2):
            cw = [wpool.tile([P, 2 * D], MM_DT, tag=f"cw{ki}") for ki in range(NC)]
            vw = [wpool.tile([P, D], MM_DT, tag=f"vw{ki}") for ki in range(NC)]
            for ki in range(NC):
                nc.sync.dma_start(out=cw[ki], in_=t["local_cW"][n, s, ki * P:(ki + 1) * P, :])
                nc.sync.dma_start(out=vw[ki], in_=t["local_vW"][n, s, ki * P:(ki + 1) * P, :])
            cbq = load_biases(t["local_cb"][n, s, 0:D], "cbq")
            vb = load_biases(t["local_vb"][n, s], "vb")
            cbv_row = bpool.tile([1, D], MM_DT, tag="cbv")
            nc.gpsimd.dma_start(out=cbv_row, in_=t["local_cb"][n, s, D:2 * D][None, :])
            for b in range(BPC):
                nl_block(b, fT[b], cw, vw, cbq, cbv_row, vb, banded=True)
        # accumulate entity result into aT, weighted by sattw[b, n]
        for b in range(BPC):
            w_ap = sattwB[:, b * NENT + n:b * NENT + n + 1]
            for c in range(NC):
                if n == 0:
                    nc.vector.tensor_scalar_mul(aT[b][c], fT[b][c], w_ap)
                else:
                    wt = rpool.tile([P, L], F32, tag="t", bufs=1)
                    nc.scalar.activation(out=wt, in_=fT[b][c], func=AF.Copy, scale=w_ap)
                    nc.vector.tensor_add(out=aT[b][c], in0=aT[b][c], in1=wt)

    # ---- global blocks ----
    for s in range(2):
        cw = [wpool.tile([P, 2 * D], MM_DT, tag=f"cw{ki}") for ki in range(NC)]
        vw = [wpool.tile([P, D], MM_DT, tag=f"vw{ki}") for ki in range(NC)]
        for ki in range(NC):
            nc.sync.dma_start(out=cw[ki], in_=t["global_cW"][s, ki * P:(ki + 1) * P, :])
            nc.sync.dma_start(out=vw[ki], in_=t["global_vW"][s, ki * P:(ki + 1) * P, :])
        cbq = load_biases(t["global_cb"][s, 0:D], "cbq")
        vb = load_biases(t["global_vb"][s], "vb")
        cbv_row = bpool.tile([1, D], MM_DT, tag="cbv")
        nc.gpsimd.dma_start(out=cbv_row, in_=t["global_cb"][s, D:2 * D][None, :])
        for b in range(BPC):
            nl_block(b, aT[b], cw, vw, cbq, cbv_row, vb, banded=False)

    # ---- transpose back and write out ----
    for b in range(BPC):
        for lc in range(NC):
            out_nat = npool.tile([P, D], F32, tag="on", bufs=1)
            for dc in range(NC):
                pst = ps_score.tile([P, P], MM_DT, tag="score")
                nc.tensor.transpose(pst, aT[b][dc][:, lc * P:(lc + 1) * P], ident_mm)
                nc.scalar.copy(out_nat[:, dc * P:(dc + 1) * P], pst)
            nc.sync.dma_start(out=t["out_a"][b, lc * P:(lc + 1) * P, :], in_=out_nat)

    ctx.close()


_NC_CACHE = None


def kernel(**inputs):
    global _NC_CACHE
    if _NC_CACHE is None:
        _NC_CACHE = _build()
    nc = _NC_CACHE

    band = _band_mask8()
    gmat_np, sel_np = _sel_gmat()
    shared = {k: np.ascontiguousarray(np.asarray(inputs[k], dtype=np.float32))
              for k in ("hp_w1", "hp_b1", "hp_w2", "hp_b2", "hp_w3", "hp_b3",
                        "local_cW", "local_cb", "local_vW", "local_vb",
                        "global_cW", "global_cb", "global_vW", "global_vb",
                        "satt_w1", "satt_w2")}
    seg_feats = np.asarray(inputs["seg_feats"], dtype=np.float32)
    seg_masks = np.asarray(inputs["seg_masks"], dtype=np.float32)
    se_feats = np.asarray(inputs["se_feats"], dtype=np.float32)

    in_maps = []
    for c in range(CORES):
        sl = slice(c * BPC, (c + 1) * BPC)
        m = dict(shared)
        m["seg_feats"] = np.ascontiguousarray(seg_feats[sl])
        m["seg_masks"] = np.ascontiguousarray(seg_masks[sl])
        m["se_feats"] = np.ascontiguousarray(se_feats[sl])
        m["band_mask8"] = band
        m["gmat_in"] = gmat_np
        m["sel_in"] = sel_np
        in_maps.append(m)

    res = bass_utils.run_bass_kernel_spmd(nc, in_maps, core_ids=list(range(CORES)))
    a = np.concatenate([res.results[c]["out_a"] for c in range(CORES)], axis=0)
    sattw = np.concatenate([res.results[c]["out_sattw"] for c in range(CORES)], axis=0)
    return a, sattw


# revision 22
# speedup vs baseline: 1.5858x; 1.0237x over previous
"""Trainium2 Bass kernel for nn_LocalGlobalVideoTextInteractions.

Data-parallel over batch: B=16 across 8 NeuronCores (2 batches/core).
All activations are kept in transposed layout fT[d, l] (feature on the
partition dim) so every projection is a natural PE matmul; attention scores
are computed directly in transposed form sT[m, l], softmax denominators come
from a ones-column appended to the value matrix, and normalization is applied
via a K=1 broadcast matmul + vector ops.

Matmul operand dtype is switchable: float32 (exact, 4 cyc/row) or float32r
(tf32-like, 1 cyc/row at N>=256).
"""
import os
import sys

import numpy as np

for _p in ("/opt/trn_rl_repo", "/root/.axon_site/_ro/trn_rl_repo"):
    if os.path.isdir(_p) and _p not in sys.path:
        sys.path.append(_p)

import concourse.bass as bass
import concourse.tile as tile
from concourse import bacc, mybir
from concourse import bass_utils

F32 = mybir.dt.float32
AF = mybir.ActivationFunctionType

B, L, D = 16, 512, 512
NENT, NH, DH = 3, 8, 64
AH = 256
CORES = 8
BPC = B // CORES  # batches per core
P = 128
NC = D // P  # 4 chunks
KW = 7  # band half-width (ksize 15, dilation 1)
NEG = -1e9
WIN = 144  # banded score window width (>= 128 + 2*KW)
LW0 = [0, 121, 249, 368]  # window start per m-chunk: clamp(128c-7, 0, 512-WIN)
BF16 = mybir.dt.bfloat16

# MM_DT: dtype of every SBUF tensor that feeds the PE array.
MM_DT = mybir.dt.float32r if os.environ.get("KERNEL_F32R", "1") == "1" else F32
BANDED = os.environ.get("KERNEL_BANDED", "0") == "1"  # (banded local path, v2)


def _band_mask8() -> np.ndarray:
    """Additive pre-scale band mask on the score windows, transposed
    orientation: tile c covers m in [128c, 128c+128) on partitions and
    l in [LW0[c], LW0[c]+WIN) on the free dim. 0 in band, -8e9 outside."""
    m = (np.arange(NC)[:, None, None] * P + np.arange(P)[None, :, None])
    l = np.asarray(LW0)[:, None, None] + np.arange(WIN)[None, None, :]
    return np.where(np.abs(l - m) <= KW, 0.0, 8.0 * NEG).astype(np.float32)


def _sel_gmat():
    gmat = np.zeros((P, 2 * P), np.float32)
    gmat[DH, P] = 1.0
    gmat[32, P] = 1.0
    sel = np.zeros((NH // 2, NH, P), np.float32)
    for jj in range(NH // 2):
        sel[jj, 2 * jj, 0:DH] = 1.0
        sel[jj, 2 * jj + 1, DH:P] = 1.0
    return gmat, sel


def _build():
    nc = bacc.Bacc("TRN2", target_bir_lowering=False, debug=False)

    def din(name, shape, dt=F32):
        return nc.dram_tensor(name, shape, dt, kind="ExternalInput").ap()

    seg_feats = din("seg_feats", [BPC, L, D])
    seg_masks = din("seg_masks", [BPC, L])
    se_feats = din("se_feats", [BPC, NENT, D], MM_DT)
    hp_w1 = din("hp_w1", [NENT, D, D], MM_DT)
    hp_b1 = din("hp_b1", [NENT, D])
    hp_w2 = din("hp_w2", [NENT, D, D], MM_DT)
    hp_b2 = din("hp_b2", [NENT, D])
    hp_w3 = din("hp_w3", [NENT, D, D], MM_DT)
    hp_b3 = din("hp_b3", [NENT, D])
    local_cW = din("local_cW", [NENT, 2, D, 2 * D], MM_DT)
    local_cb = din("local_cb", [NENT, 2, 2 * D])
    local_vW16 = din("local_vW16", [NENT, 2, D, D], BF16)
    local_cWq16 = din("local_cWq16", [NENT, 2, D, D], BF16)
    local_vb = din("local_vb", [NENT, 2, D])
    global_cW = din("global_cW", [2, D, 2 * D], MM_DT)
    global_cb = din("global_cb", [2, 2 * D])
    global_vW16 = din("global_vW16", [2, D, D], BF16)
    global_cWq16 = din("global_cWq16", [2, D, D], BF16)
    global_vb = din("global_vb", [2, D])
    satt_w1 = din("satt_w1", [D, AH], MM_DT)
    satt_w2 = din("satt_w2", [AH, 1], MM_DT)
    band_mask8 = din("band_mask8", [NC, P, WIN])
    gmat_in = din("gmat_in", [P, 2 * P], MM_DT)
    sel_in = din("sel_in", [NH // 2, NH, P], MM_DT)

    out_a = nc.dram_tensor("out_a", [BPC, L, D], F32, kind="ExternalOutput").ap()
    out_sattw = nc.dram_tensor("out_sattw", [BPC, NENT], F32, kind="ExternalOutput").ap()

    with tile.TileContext(nc) as tc:
        _emit(nc, tc, locals())
    nc.compile()
    return nc


def _autoname(pool):
    orig = pool.tile

    def tile(shape, dtype, **kw):
        if "name" not in kw:
            kw["name"] = kw.get("tag") or "tmp"
        return orig(shape, dtype, **kw)

    pool.tile = tile
    return pool


def _emit(nc, tc, t):
    from contextlib import ExitStack

    ctx = ExitStack()
    const = ctx.enter_context(tc.tile_pool(name="const", bufs=1))
    wpool = ctx.enter_context(tc.tile_pool(name="wpool", bufs=2))
    bpool = ctx.enter_context(tc.tile_pool(name="bpool", bufs=2))
    fpool = ctx.enter_context(tc.tile_pool(name="fpool", bufs=1))
    apool = ctx.enter_context(tc.tile_pool(name="apool", bufs=1))
    spool = ctx.enter_context(tc.tile_pool(name="spool", bufs=1))  # segT
    hpool = ctx.enter_context(tc.tile_pool(name="hpool", bufs=1))  # h1
    kpool = ctx.enter_context(tc.tile_pool(name="kpool", bufs=2))  # mk/mq
    vpool = ctx.enter_context(tc.tile_pool(name="vpool", bufs=1))  # mv_aug
    rpool = ctx.enter_context(tc.tile_pool(name="rpool", bufs=2))  # resid temps
    npool = ctx.enter_context(tc.tile_pool(name="npool", bufs=2))  # nat in/out

    ps_proj = ctx.enter_context(tc.tile_pool(name="ps_proj", bufs=2, space="PSUM"))
    ps_score = ctx.enter_context(tc.tile_pool(name="ps_score", bufs=2, space="PSUM"))
    ps_av = ctx.enter_context(tc.tile_pool(name="ps_av", bufs=2, space="PSUM"))
    ps_bc = ctx.enter_context(tc.tile_pool(name="ps_bc", bufs=2, space="PSUM"))

    for _pl in (const, wpool, bpool, fpool, apool, spool, hpool, kpool, vpool,
                rpool, npool, ps_proj, ps_score, ps_av, ps_bc):
        _autoname(_pl)

    mm = nc.tensor.matmul

    # ---- constants ----
    ones_mat = const.tile([P, P], MM_DT, tag="ones")
    nc.vector.memset(ones_mat.bitcast(F32), 1.0)
    ident = const.tile([P, P], F32, tag="ident")
    from concourse.masks import make_identity
    make_identity(nc, ident)
    if MM_DT != F32:
        ident_mm = const.tile([P, P], MM_DT, tag="identmm")
        nc.vector.tensor_copy(ident_mm, ident)
    else:
        ident_mm = ident

    # gmat[row, 128] = 1 for row in {32, 64}: K=1 gather matmuls slide the
    # free-dim window so head h's denominator lands on psum row h.
    # sel_j [8, 128]: row 2j -> partitions 0:64, row 2j+1 -> 64:128.
    gmat = const.tile([P, 2 * P], MM_DT, tag="gmat")
    nc.sync.dma_start(out=gmat, in_=t["gmat_in"])
    sel = []
    for jj in range(NH // 2):
        s_t = const.tile([NH, P], MM_DT, tag=f"sel{jj}")
        nc.sync.dma_start(out=s_t, in_=t["sel_in"][jj])
        sel.append(s_t)

    band_sb = [const.tile([P, WIN], F32, tag=f"band{c}") for c in range(NC)]
    for c in range(NC):
        nc.sync.dma_start(out=band_sb[c], in_=t["band_mask8"][c])

    # persistent exp(score) tiles, double-buffered by head parity:
    # outside-window region stays zero forever
    eTt = [[const.tile([P, L], MM_DT, tag=f"e{c}_{p}") for p in range(2)]
           for c in range(NC)]
    for c in range(NC):
        for p in range(2):
            nc.vector.memset(eTt[c][p].bitcast(F32), 0.0)

    # seg mask -> additive bias per m-partition: (mask-1)*1e9
    negb = const.tile([P, 1], F32, tag="negb")
    nc.vector.memset(negb, NEG)
    mbias = [[const.tile([P, 1], F32, tag=f"mb{b}{c}") for c in range(NC)]
             for b in range(BPC)]
    for b in range(BPC):
        for c in range(NC):
            nc.sync.dma_start(out=mbias[b][c], in_=t["seg_masks"][b, c * P:(c + 1) * P])
            nc.scalar.activation(out=mbias[b][c], in_=mbias[b][c], func=AF.Identity,
                                 scale=-NEG, bias=negb)

    # seT[ki][p, b, n] = se_feats[b, n, 128ki + p]
    seT = [const.tile([P, BPC, NENT], MM_DT, tag=f"seT{ki}") for ki in range(NC)]
    for ki in range(NC):
        ap = bass.AP(tensor=t["se_feats"].tensor, offset=ki * P,
                     ap=[[1, P], [NENT * D, BPC], [D, NENT]])
        nc.sync.dma_start(out=seT[ki], in_=ap)

    # ---- attentive pooling (se_feats only) -> sattwB [128, (b,n)] ----
    sw1 = [const.tile([P, AH], MM_DT, tag=f"sw1{ki}") for ki in range(NC)]
    for ki in range(NC):
        nc.sync.dma_start(out=sw1[ki], in_=t["satt_w1"][ki * P:(ki + 1) * P, :])
    sw2 = [const.tile([P, 1], MM_DT, tag=f"sw2{a}") for a in range(AH // P)]
    for a in range(AH // P):
        nc.sync.dma_start(out=sw2[a], in_=t["satt_w2"][a * P:(a + 1) * P, :])

    th = [const.tile([P, BPC * NENT], MM_DT, tag=f"th{a}") for a in range(AH // P)]
    for a in range(AH // P):
        ps = ps_proj.tile([P, BPC * NENT], F32, tag="proj")
        for ki in range(NC):
            mm(ps, sw1[ki][:, a * P:(a + 1) * P],
               seT[ki].rearrange("p b n -> p (b n)"),
               start=(ki == 0), stop=(ki == NC - 1))
        nc.scalar.activation(out=th[a], in_=ps, func=AF.Tanh)
    ps_alpha = ps_av.tile([1, BPC * NENT], F32, tag="av")
    for a in range(AH // P):
        mm(ps_alpha, sw2[a], th[a], start=(a == 0), stop=(a == AH // P - 1))
    ealpha = const.tile([1, BPC * NENT], F32, tag="ealpha")
    # magnitudes are tiny: skip max-subtraction in these softmaxes
    nc.scalar.activation(out=ealpha, in_=ps_alpha, func=AF.Exp)
    asum = const.tile([1, BPC], F32, tag="asum")
    nc.vector.reduce_sum(out=asum, in_=ealpha.rearrange("o (b n) -> o b n", b=BPC),
                         axis=mybir.AxisListType.X)
    arecip = const.tile([1, BPC], F32, tag="arecip")
    nc.vector.reciprocal(arecip, asum)
    sattw = const.tile([1, BPC * NENT], MM_DT, tag="sattw")
    for b in range(BPC):
        nc.vector.tensor_scalar_mul(sattw[:, b * NENT:(b + 1) * NENT],
                                    ealpha[:, b * NENT:(b + 1) * NENT],
                                    arecip[:, b:b + 1])
    nc.gpsimd.dma_start(out=t["out_sattw"].rearrange("b n -> (b n)")[None, :], in_=sattw)
    ps_sw = ps_bc.tile([P, BPC * NENT], F32, tag="bc")
    mm(ps_sw, ones_mat[0:1, :], sattw, start=True, stop=True)
    sattwB = const.tile([P, BPC * NENT], F32, tag="sattwB")
    nc.scalar.copy(sattwB, ps_sw)

    # ---- h2[n] = relu(se_feats[:, n] @ hp_w2[n] + hp_b2[n]), all (b, n) ----
    h2T = [[const.tile([P, BPC], F32, tag=f"h2{n}{c}") for c in range(NC)]
           for n in range(NENT)]
    for n in range(NENT):
        w2 = [wpool.tile([P, D], MM_DT, tag=f"w1_{ki}", bufs=1) for ki in range(NC)]
        for ki in range(NC):
            nc.sync.dma_start(out=w2[ki], in_=t["hp_w2"][n, ki * P:(ki + 1) * P, :])
        for c in range(NC):
            b2 = bpool.tile([P, 1], F32, tag="b2")
            nc.sync.dma_start(out=b2, in_=t["hp_b2"][n, c * P:(c + 1) * P])
            ps = ps_proj.tile([P, BPC], F32, tag="proj")
            for ki in range(NC):
                mm(ps, w2[ki][:, c * P:(c + 1) * P], seT[ki][:, :, n],
                   start=(ki == 0), stop=(ki == NC - 1))
            nc.scalar.activation(out=h2T[n][c], in_=ps, func=AF.Relu, bias=b2)

    # ---- transpose seg_feats -> segT[b][dc] [128(d), 512(l)] ----
    segT = [[spool.tile([P, L], MM_DT, tag=f"seg{b}{dc}") for dc in range(NC)]
            for b in range(BPC)]
    for b in range(BPC):
        for lc in range(NC):
            nat = npool.tile([P, D], F32, tag="nat", bufs=1)
            nc.sync.dma_start(out=nat, in_=t["seg_feats"][b, lc * P:(lc + 1) * P, :])
            for dc in range(NC):
                pst = ps_score.tile([P, P], F32, tag="score")
                nc.tensor.transpose(pst, nat[:, dc * P:(dc + 1) * P], ident)
                nc.scalar.copy(segT[b][dc][:, lc * P:(lc + 1) * P], pst)

    fT = [[fpool.tile([P, L], MM_DT, tag=f"f{b}{c}") for c in range(NC)]
          for b in range(BPC)]
    aT = [[apool.tile([P, L], MM_DT, tag=f"a{b}{c}") for c in range(NC)]
          for b in range(BPC)]

    def load_biases(src_row, tag):
        """4x [128,1] f32 bias tiles from a length-512 DRAM row."""
        tiles = []
        for c in range(NC):
            bt = bpool.tile([P, 1], F32, tag=f"{tag}{c}")
            nc.sync.dma_start(out=bt, in_=src_row[c * P:(c + 1) * P])
            tiles.append(bt)
        return tiles

    def nl_block(b, xT, cwq, cwv, vw, cbq, cbv_row, vb, banded):
        """One non-local block, in-place residual update of xT (4 tiles)."""
        # bf16 shadow of the activations: mk/mq feed only the softmax scores,
        # which are O(1e-5) here, so bf16 projections are ample
        xT16 = [kpool.tile([P, L], BF16, tag=f"x16{c}", bufs=1) for c in range(NC)]
        for c in range(NC):
            nc.vector.tensor_copy(xT16[c], xT[c])
        mk = [kpool.tile([P, L], BF16, tag=f"mk{c}") for c in range(NC)]
        mq = [kpool.tile([P, L], BF16, tag=f"mq{c}") for c in range(NC)]
        for c in range(NC):
            ps = ps_proj.tile([P, L], F32, tag="proj")
            for ki in range(NC):
                mm(ps, vw[ki][:, c * P:(c + 1) * P], xT16[ki],
                   start=(ki == 0), stop=(ki == NC - 1))
            nc.scalar.activation(out=mk[c], in_=ps, func=AF.Identity, bias=vb[c])
            ps = ps_proj.tile([P, L], F32, tag="proj")
            for ki in range(NC):
                mm(ps, cwq[ki][:, c * P:(c + 1) * P], xT16[ki],
                   start=(ki == 0), stop=(ki == NC - 1))
            nc.scalar.activation(out=mq[c], in_=ps, func=AF.Identity, bias=cbq[c])
        # mv in natural layout [m, dv], embedded as the stationary operand of
        # the AV matmul so each head's result lands on its own psum partitions
        # (trn2 matmul output always starts at partition 0): even heads use
        # columns 0:64 + ones at 64; odd heads columns 64:128 + ones at 32.
        # Unused columns are zero.
        mv = [vpool.tile([P, NH, P], MM_DT, tag=f"mv{c}") for c in range(NC)]
        for c in range(NC):
            ps = ps_proj.tile([P, L], F32, tag="proj")
            for ki in range(NC):
                mm(ps, xT[ki][:, c * P:(c + 1) * P], cwv[ki],
                   start=(ki == 0), stop=False)
            mm(ps, ones_mat[0:1, :], cbv_row, start=False, stop=True)
            psv = ps[:].rearrange("p (h d) -> p h d", h=NH)
            nc.gpsimd.memset(mv[c].bitcast(F32), 0.0)
            nc.vector.tensor_copy(mv[c][:, 0::2, 0:DH], psv[:, 0::2, :])
            nc.vector.tensor_copy(mv[c][:, 1::2, DH:P], psv[:, 1::2, :])
            nc.gpsimd.memset(mv[c][:, 0::2, DH:DH + 1].bitcast(F32), 1.0)
            nc.gpsimd.memset(mv[c][:, 1::2, 32:33].bitcast(F32), 1.0)
        # per-head scores + AV; denominators are gathered onto rows 0..7 of
        # one psum tile so a single reciprocal serves the whole block, and
        # each head's rT slice is copied to a per-pair SBUF tile (its psum
        # bank is then free for the next head).
        stack_ps = ps_bc.tile([P, L], F32, tag="bc")
        rts = []
        for h in range(NH):
            j, base = h // 2, (h % 2) * DH
            for c in range(NC):
                if banded:
                    ps = ps_score.tile([P, WIN], F32, tag="score")
                    mm(ps, mq[j][base:base + DH, c * P:(c + 1) * P],
                       mk[j][base:base + DH, LW0[c]:LW0[c] + WIN],
                       start=True, stop=True)
                    nc.vector.tensor_add(out=ps, in0=ps, in1=band_sb[c])
                    nc.scalar.activation(out=eTt[c][h % 2][:, LW0[c]:LW0[c] + WIN],
                                         in_=ps, func=AF.Exp,
                                         scale=1.0 / np.sqrt(DH), bias=mbias[b][c])
                else:
                    ps = ps_score.tile([P, L], F32, tag="score")
                    mm(ps, mq[j][base:base + DH, c * P:(c + 1) * P],
                       mk[j][base:base + DH, :], start=True, stop=True)
                    nc.scalar.activation(out=eTt[c][h % 2], in_=ps, func=AF.Exp,
                                         scale=1.0 / np.sqrt(DH), bias=mbias[b][c])
            # AV: the head-parity column placement in mv makes rT land on
            # the head's own partitions; denominator on a spare aligned row.
            psr = ps_av.tile([P, L], F32, tag="av")
            for c in range(NC):
                mm(psr, mv[c][:, h, :], eTt[c][h % 2], start=(c == 0), stop=(c == NC - 1))
            row = DH if h % 2 == 0 else 32
            if h % 2 == 0:
                rt = rpool.tile([P, L], F32, tag=f"rt{j % 2}")
                rts.append(rt)
            else:
                rt = rts[j]
            nc.vector.tensor_copy(rt[base:base + DH, :], psr[base:base + DH, :])
            dnr = rpool.tile([P, L], MM_DT, tag="dnr", bufs=1)
            nc.scalar.copy(dnr[row:row + 1, :], psr[row:row + 1, :])
            mm(stack_ps, gmat[row:row + 1, P - h:2 * P - h], dnr[row:row + 1, :],
               start=(h == 0), stop=(h == NH - 1))
        rc8 = rpool.tile([NH, L], F32, tag="recip", bufs=1)
        nc.vector.reciprocal_approx_fast(out=rc8, in_=stack_ps[0:NH, :])
        rc8r = rpool.tile([NH, L], MM_DT, tag="recipr", bufs=1)
        nc.scalar.copy(rc8r, rc8)
        for jj in range(NH // 2):
            psb = ps_bc.tile([P, L], F32, tag="bc")
            mm(psb, sel[jj], rc8r, start=True, stop=True)
            tt = rpool.tile([P, L], F32, tag="t", bufs=1)
            nc.vector.tensor_mul(tt, rts[jj], psb)
            nc.vector.tensor_add(out=xT[jj], in0=xT[jj], in1=tt)

    # ---- main pipeline over entities ----
    for n in range(NENT):
        w1 = [wpool.tile([P, D], MM_DT, tag=f"w1_{ki}", bufs=1) for ki in range(NC)]
        w3 = [wpool.tile([P, D], MM_DT, tag=f"w3_{ki}", bufs=1) for ki in range(NC)]
        for ki in range(NC):
            nc.sync.dma_start(out=w1[ki], in_=t["hp_w1"][n, ki * P:(ki + 1) * P, :])
            nc.sync.dma_start(out=w3[ki], in_=t["hp_w3"][n, ki * P:(ki + 1) * P, :])
        b1 = load_biases(t["hp_b1"][n], "b1")
        b3 = load_biases(t["hp_b3"][n], "b3")
        for b in range(BPC):
            h1 = [hpool.tile([P, L], MM_DT, tag=f"h1{c}") for c in range(NC)]
            for c in range(NC):
                ps = ps_proj.tile([P, L], F32, tag="proj")
                for ki in range(NC):
                    mm(ps, w1[ki][:, c * P:(c + 1) * P], segT[b][ki],
                       start=(ki == 0), stop=(ki == NC - 1))
                nc.scalar.activation(out=h1[c], in_=ps, func=AF.Relu, bias=b1[c])
                nc.vector.tensor_scalar_mul(h1[c], h1[c], h2T[n][c][:, b:b + 1])
            for c in range(NC):
                ps = ps_proj.tile([P, L], F32, tag="proj")
                for ki in range(NC):
                    mm(ps, w3[ki][:, c * P:(c + 1) * P], h1[ki],
                       start=(ki == 0), stop=(ki == NC - 1))
                nc.scalar.activation(out=fT[b][c], in_=ps, func=AF.Relu, bias=b3[c])
        for s in range(2):
            cwq = [wpool.tile([P, D], BF16, tag=f"cwq{ki}") for ki in range(NC)]
            cwv = [wpool.tile([P, D], MM_DT, tag=f"cwv{ki}") for ki in range(NC)]
            vw = [wpool.tile([P, D], BF16, tag=f"vw{ki}") for ki in range(NC)]
            for ki in range(NC):
                nc.sync.dma_start(out=cwq[ki], in_=t["local_cWq16"][n, s, ki * P:(ki + 1) * P, :])
                nc.sync.dma_start(out=cwv[ki], in_=t["local_cW"][n, s, ki * P:(ki + 1) * P, D:2 * D])
                nc.sync.dma_start(out=vw[ki], in_=t["local_vW16"][n, s, ki * P:(ki + 1) * P, :])
            cbq = load_biases(t["local_cb"][n, s, 0:D], "cbq")
            vb = load_biases(t["local_vb"][n, s], "vb")
            cbv_row = bpool.tile([1, D], MM_DT, tag="cbv")
            nc.gpsimd.dma_start(out=cbv_row, in_=t["local_cb"][n, s, D:2 * D][None, :])
            for b in range(BPC):
                nl_block(b, fT[b], cwq, cwv, vw, cbq, cbv_row, vb, banded=True)
        # accumulate entity result into aT, weighted by sattw[b, n]
        for b in range(BPC):
            w_ap = sattwB[:, b * NENT + n:b * NENT + n + 1]
            for c in range(NC):
                if n == 0:
                    nc.vector.tensor_scalar_mul(aT[b][c], fT[b][c], w_ap)
                else:
                    wt = rpool.tile([P, L], F32, tag="t", bufs=1)
                    nc.scalar.activation(out=wt, in_=fT[b][c], func=AF.Copy, scale=w_ap)
                    nc.vector.tensor_add(out=aT[b][c], in0=aT[b][c], in1=wt)

    # ---- global blocks ----
    for s in range(2):
        cwq = [wpool.tile([P, D], BF16, tag=f"cwq{ki}") for ki in range(NC)]
        cwv = [wpool.tile([P, D], MM_DT, tag=f"cwv{ki}") for ki in range(NC)]
        vw = [wpool.tile([P, D], BF16, tag=f"vw{ki}") for ki in range(NC)]
        for ki in range(NC):
            nc.sync.dma_start(out=cwq[ki], in_=t["global_cWq16"][s, ki * P:(ki + 1) * P, :])
            nc.sync.dma_start(out=cwv[ki], in_=t["global_cW"][s, ki * P:(ki + 1) * P, D:2 * D])
            nc.sync.dma_start(out=vw[ki], in_=t["global_vW16"][s, ki * P:(ki + 1) * P, :])
        cbq = load_biases(t["global_cb"][s, 0:D], "cbq")
        vb = load_biases(t["global_vb"][s], "vb")
        cbv_row = bpool.tile([1, D], MM_DT, tag="cbv")
        nc.gpsimd.dma_start(out=cbv_row, in_=t["global_cb"][s, D:2 * D][None, :])
        for b in range(BPC):
            nl_block(b, aT[b], cwq, cwv, vw, cbq, cbv_row, vb, banded=False)

    # ---- transpose back and write out ----
    for b in range(BPC):
        for lc in range(NC):
            out_nat = npool.tile([P, D], F32, tag="on", bufs=1)
            for dc in range(NC):
                pst = ps_score.tile([P, P], MM_DT, tag="score")
                nc.tensor.transpose(pst, aT[b][dc][:, lc * P:(lc + 1) * P], ident_mm)
                nc.scalar.copy(out_nat[:, dc * P:(dc + 1) * P], pst)
            nc.sync.dma_start(out=t["out_a"][b, lc * P:(lc + 1) * P, :], in_=out_nat)

    ctx.close()


_NC_CACHE = None


def kernel(**inputs):
    global _NC_CACHE
    if _NC_CACHE is None:
        _NC_CACHE = _build()
    nc = _NC_CACHE

    band = _band_mask8()
    gmat_np, sel_np = _sel_gmat()
    import ml_dtypes
    shared = {k: np.ascontiguousarray(np.asarray(inputs[k], dtype=np.float32))
              for k in ("hp_w1", "hp_b1", "hp_w2", "hp_b2", "hp_w3", "hp_b3",
                        "local_cW", "local_cb", "local_vb",
                        "global_cW", "global_cb", "global_vb",
                        "satt_w1", "satt_w2")}
    bf = ml_dtypes.bfloat16
    shared["local_vW16"] = np.ascontiguousarray(
        np.asarray(inputs["local_vW"], np.float32).astype(bf))
    shared["local_cWq16"] = np.ascontiguousarray(
        np.asarray(inputs["local_cW"], np.float32)[:, :, :, 0:D].astype(bf))
    shared["global_vW16"] = np.ascontiguousarray(
        np.asarray(inputs["global_vW"], np.float32).astype(bf))
    shared["global_cWq16"] = np.ascontiguousarray(
        np.asarray(inputs["global_cW"], np.float32)[:, :, 0:D].astype(bf))
    seg_feats = np.asarray(inputs["seg_feats"], dtype=np.float32)
    seg_masks = np.asarray(inputs["seg_masks"], dtype=np.float32)
    se_feats = np.asarray(inputs["se_feats"], dtype=np.float32)

    in_maps = []
    for c in range(CORES):
        sl = slice(c * BPC, (c + 1) * BPC)
        m = dict(shared)
        m["seg_feats"] = np.ascontiguousarray(seg_feats[sl])
        m["seg_masks"] = np.ascontiguousarray(seg_masks[sl])
        m["se_feats"] = np.ascontiguousarray(se_feats[sl])
        m["band_mask8"] = band
        m["gmat_in"] = gmat_np
        m["sel_in"] = sel_np
        in_maps.append(m)

    res = bass_utils.run_bass_kernel_spmd(nc, in_maps, core_ids=list(range(CORES)))
    a = np.concatenate([res.results[c]["out_a"] for c in range(CORES)], axis=0)
    sattw = np.concatenate([res.results[c]["out_sattw"] for c in range(CORES)], axis=0)
    return a, sattw
